# revision 2
# baseline (speedup 1.0000x reference)
"""Trainium2 Bass kernel for a dense transformer encoder layer (v2).

Reference semantics (B=2, S=2048, D=1024, H=16, DH=64, HID=4096):
    q = einsum('bsd,hde->bhse', x, Wq) + bq          (q == k == v, source bug)
    prob = softmax(q @ q^T / sqrt(DH))
    attn = concat_heads(prob @ q)
    x1 = LN(x + attn);  ff = relu(x1 @ W1 + b1) @ W2 + b2;  out = LN(x1 + ff)

Sharding: 8 cores, core c -> batch b=c//4, token quarter t=c%4.  Each core
computes q for the full (rotated) sequence of its batch, then attention +
FFN for its own 512 tokens.  Zero collectives; quarters reassembled on host.

Structure:
  - Host supplies x^T pre-packed in fp8 DoubleRow layout; q projection and
    both FFN matmuls run fp8e4 DoubleRow (weights pre-scaled x16, unscaled
    in the psum epilogues; h1 is kept at 16x so relu needs no extra op).
  - q-natural layout (wv moving operand) comes from on-chip PE transposes
    of qT; no DMA transposes or DRAM round trips anywhere.
  - wv: stationary = exp(scores) [keys x queries] chunk, moving = q-natural
    [keys x 64] plus a ones column for the softmax denominator, so head
    outputs land directly in [queries, dims] layout; epilogue is one
    reciprocal + scalar_tensor_tensor into y1 per (h, qc).
  - exp(scores/8 - 1) keeps E inside fp8e4 range; softmax cancels e^-1.
  - The ACT engine (exp) is the attention bottleneck; scores pace it via
    psum rotation while wv/qproj/transpose chunks fill PE between score
    matmuls.  DMAs are scheduled so xT/Wq land first and W1/W2 stream in
    mid-attention.  LayerNorm stats use bn_stats/bn_aggr on DVE.
"""

import numpy as np

import concourse.bacc as bacc
import concourse.mybir as mybir
from concourse import tile
from concourse.bass_utils import run_bass_kernel_spmd

dt = mybir.dt
AF = mybir.ActivationFunctionType
ALU = mybir.AluOpType
DR = mybir.MatmulPerfMode.DoubleRow

B, S, D = 2, 2048, 1024
H, DH, HID = 16, 64, 4096
SQ = S // 4
NCORES = 8
EPS = 1e-5
WS = 16.0          # fp8 weight pre-scale
F32, BF16, F8 = dt.float32, dt.bfloat16, dt.float8e4

_BUILD_CACHE = {}


def _build(apply_affine: bool):
    if apply_affine in _BUILD_CACHE:
        return _BUILD_CACHE[apply_affine]

    nc = bacc.Bacc("TRN2", target_bir_lowering=False, debug=False,
                   num_devices=NCORES)

    xT_dr = nc.dram_tensor("xT_dr", [4, 128, 2, S], F8,
                           kind="ExternalInput").ap()
    x_q = nc.dram_tensor("x_q", [SQ, D], F32, kind="ExternalInput").ap()
    wq_dr = nc.dram_tensor("wq_dr", [4, 128, 2, D], F8,
                           kind="ExternalInput").ap()
    bq_r = nc.dram_tensor("bq_r", [128, 8], F32, kind="ExternalInput").ap()
    w1_dr = nc.dram_tensor("w1_dr", [4, 128, 2, HID], F8,
                           kind="ExternalInput").ap()
    b1_r = nc.dram_tensor("b1_r", [128, 32], F32, kind="ExternalInput").ap()
    w2_dr = nc.dram_tensor("w2_dr", [16, 128, 2, D], F8,
                           kind="ExternalInput").ap()
    b2_bc = nc.dram_tensor("b2_bc", [128, D], F32, kind="ExternalInput").ap()
    if apply_affine:
        g1d = nc.dram_tensor("g1d", [128, D], F32, kind="ExternalInput").ap()
        be1d = nc.dram_tensor("be1d", [128, D], F32,
                              kind="ExternalInput").ap()
        g2d = nc.dram_tensor("g2d", [128, D], F32, kind="ExternalInput").ap()
        be2d = nc.dram_tensor("be2d", [128, D], F32,
                              kind="ExternalInput").ap()
    out_q = nc.dram_tensor("out_q", [SQ, D], F32, kind="ExternalOutput").ap()

    with tile.TileContext(nc) as tc:
        with (
            tc.tile_pool(name="const", bufs=1) as cpool,
            tc.tile_pool(name="wts", bufs=1) as wpool,
            tc.tile_pool(name="y1", bufs=4) as y1pool,
            tc.tile_pool(name="x1", bufs=1) as x1pool,
            tc.tile_pool(name="ln", bufs=2) as lnpool,
        ):
            # ---- small constants (cheap DMAs first, they are tiny) ----
            bq_sb = cpool.tile([128, 8], F32)
            nc.sync.dma_start(bq_sb[:], bq_r[:])
            b1_sb = cpool.tile([128, 32], F32)
            nc.sync.dma_start(b1_sb[:], b1_r[:])

            eps_sb = cpool.tile([128, 1], F32)
            nc.vector.memset(eps_sb[:], EPS)
            neg1_sb = cpool.tile([128, 1], F32)
            nc.vector.memset(neg1_sb[:], -1.0)
            ones_f8 = cpool.tile([128, 1], F8)
            nc.vector.memset(ones_f8[:], 1.0)

            # fp8 identity for PE-mode transposes
            col_i = cpool.tile([128, 128], F32)
            nc.gpsimd.iota(col_i[:], [[1, 128]], channel_multiplier=0,
                           allow_small_or_imprecise_dtypes=True)
            row_i = cpool.tile([128, 1], F32)
            nc.gpsimd.iota(row_i[:], [[0, 1]], channel_multiplier=1,
                           allow_small_or_imprecise_dtypes=True)
            idn = cpool.tile([128, 128], BF16)
            nc.vector.tensor_scalar(idn[:], col_i[:], row_i[:, 0:1], None,
                                    ALU.is_equal)

            b2_sb = cpool.tile([128, D], F32)
            y1s = [y1pool.tile([128, D], F32, tag="y1", name=f"y1_{sub}")
                   for sub in range(4)]
            w1_sb = [wpool.tile([128, 2, HID], F8, tag=f"w1{k}",
                                name=f"w1_{k}") for k in range(4)]
            w2_sb = [wpool.tile([128, 2, D], F8, tag=f"w2{j}",
                                name=f"w2_{j}") for j in range(16)]
            if apply_affine:
                g1_sb = cpool.tile([128, D], F32)
                nc.sync.dma_start(g1_sb[:], g1d[:])
                be1_sb = cpool.tile([128, D], F32)
                nc.sync.dma_start(be1_sb[:], be1d[:])
                g2_sb = cpool.tile([128, D], F32)
                nc.sync.dma_start(g2_sb[:], g2d[:])
                be2_sb = cpool.tile([128, D], F32)
                nc.sync.dma_start(be2_sb[:], be2d[:])

            with (
                tc.tile_pool(name="qT", bufs=1) as qTpool,
                tc.tile_pool(name="qa", bufs=12) as qapool,
                tc.tile_pool(name="E", bufs=18) as Epool,
                tc.tile_pool(name="xw", bufs=1) as xwpool,
                tc.tile_pool(name="qps", bufs=1, space="PSUM") as qps,
                tc.tile_pool(name="scps", bufs=2, space="PSUM") as scps,
                tc.tile_pool(name="tps", bufs=1, space="PSUM") as tps,
                tc.tile_pool(name="uvps", bufs=2, space="PSUM") as uvps,
            ):
                # xT / Wq first: they gate the whole pipeline
                wq_sb = []
                xT_sb = []
                for k in range(4):
                    t = xwpool.tile([128, 2, D], F8, tag=f"wq{k}",
                                    name=f"wq_{k}")
                    nc.sync.dma_start(t[:, :, 0:128], wq_dr[k][:, :, 0:128])
                    wq_sb.append(t)
                    t = xwpool.tile([128, 2, S], F8, tag=f"xT{k}",
                                    name=f"xT_{k}")
                    nc.sync.dma_start(t[:, :, 0:512], xT_dr[k][:, :, 0:512])
                    xT_sb.append(t)
                for k in range(4):
                    nc.sync.dma_start(xT_sb[k][:, :, 512:S],
                                      xT_dr[k][:, :, 512:S])
                for k in range(4):
                    nc.sync.dma_start(wq_sb[k][:, :, 128:D],
                                      wq_dr[k][:, :, 128:D])
                qT = [qTpool.tile([128, S], BF16, tag=f"qT{p}",
                                  name=f"qT{p}") for p in range(8)]
                qa = [[None] * 4 for _ in range(8)]

                def emit_qproj(p):
                    """qT[p] = (x @ Wq[:, pair p])^T via DR matmuls + bias."""
                    for n in range(4):
                        ps = qps.tile([128, 512], F32, tag="qps",
                                      name=f"qps{p}_{n}")
                        for k in range(4):
                            nc.tensor.matmul(
                                ps[:],
                                wq_sb[k][:, :, p * 128:(p + 1) * 128],
                                xT_sb[k][:, :, n * 512:(n + 1) * 512],
                                start=(k == 0), stop=(k == 3), perf_mode=DR)
                        nc.vector.tensor_scalar(
                            qT[p][:, n * 512:(n + 1) * 512], ps[:],
                            1.0 / WS, bq_sb[:, p:p + 1], ALU.mult, ALU.add)

                def emit_qtrans(p):
                    """qa[p][g]: 4 chunks x [one|h0 d64|one|h1 d64] via PE
                    transpose.  The leading ones give the softmax
                    denominator as row 0 of the wv output."""
                    for g in range(4):
                        pt = tps.tile([128, 512], BF16, tag="tps",
                                      name=f"tps{p}_{g}")
                        for j in range(4):
                            c = 4 * g + j
                            nc.tensor.transpose(
                                pt[:, j * 128:(j + 1) * 128],
                                qT[p][:, c * 128:(c + 1) * 128], idn[:])
                        t = qapool.tile([128, 4, 2, 65], F8, tag="qa",
                                        name=f"qa{p}_{g}")
                        nc.gpsimd.memset(t[:, :, :, 0:1], 1.0)
                        nc.vector.tensor_copy(
                            t[:, :, :, 1:65],
                            pt[:].rearrange("p (c h d) -> p c h d",
                                            c=4, h=2))
                        qa[p][g] = t

                E = {}      # (h, kp) -> fp8 [128, 1024] tile

                def emit_scores_kp(h, kp):
                    p, half = h // 2, h % 2
                    rows = slice(half * 64, half * 64 + 64)
                    sc = scps.tile([128, 1024], F32, tag="sc",
                                   name=f"sc{h}_{kp}")
                    for kk in range(2):
                        c = 2 * kp + kk
                        nc.tensor.matmul(
                            sc[:, kk * 512:(kk + 1) * 512],
                            qT[p][rows, c * 128:(c + 1) * 128],
                            qT[p][rows, 0:512],
                            start=True, stop=True)
                    e = Epool.tile([128, 1024], F8, tag="E",
                                   name=f"E{h}_{kp}")
                    nc.scalar.activation(e[:], sc[:], AF.Exp, scale=0.125,
                                         bias=neg1_sb[:, 0:1])
                    E[(h, kp)] = e

                def emit_wv_qc(h, qc):
                    p, half = h // 2, h % 2
                    uv = uvps.tile([128, 65], F32, tag="uv",
                                   name=f"uv{h}_{qc}")
                    for kc in range(16):
                        est = E[(h, kc // 2)][
                            :, (kc % 2) * 512 + qc * 128:
                               (kc % 2) * 512 + (qc + 1) * 128]
                        qam = qa[p][kc // 4][:, kc % 4, half, :]
                        nc.tensor.matmul(uv[:], est, qam,
                                         start=(kc == 0), stop=(kc == 15))
                    rct = lnpool.tile([128, 1], F32, tag="rct", bufs=4,
                                      name=f"rct{h}_{qc}")
                    nc.vector.reciprocal(rct[:], uv[:, 0:1])
                    nc.vector.scalar_tensor_tensor(
                        y1s[qc][:, h * 64:(h + 1) * 64],
                        uv[:, 1:65], rct[:, 0:1],
                        y1s[qc][:, h * 64:(h + 1) * 64],
                        ALU.mult, ALU.add)

                # ---- attention pipeline, ACT(exp)-paced ----
                emit_qproj(0)
                emit_qtrans(0)
                # residual rows + FFN consts: after xT/wq on the DMA queue
                for sub in range(4):
                    nc.sync.dma_start(y1s[sub][:],
                                      x_q[sub * 128:(sub + 1) * 128, :])
                nc.sync.dma_start(b2_sb[:], b2_bc[:])

                for h in range(16):
                    nxt = h // 2 + 1
                    for kp in range(8):
                        emit_scores_kp(h, kp)
                        if h >= 1 and 1 <= kp <= 4:
                            emit_wv_qc(h - 1, kp - 1)
                        if h % 2 == 0 and nxt < 8:
                            if kp == 5:
                                emit_qproj(nxt)
                            elif kp == 6:
                                emit_qtrans(nxt)
                    if h == 5:
                        for k in range(4):
                            nc.sync.dma_start(w1_sb[k][:], w1_dr[k])
                    if h == 9:
                        for j in range(16):
                            nc.sync.dma_start(w2_sb[j][:], w2_dr[j])
                for qc in range(4):
                    emit_wv_qc(15, qc)

            # ---- LN1 -> x1 (fp8 + f32+b2 variants) ----
            x1f8 = []
            x1pb = []
            for sub in range(4):
                mv, rstd, nmr = _ln_mv(nc, lnpool, y1s[sub], eps_sb)
                f8 = x1pool.tile([128, D], BF16, tag=f"x1f8_{sub}",
                                 name=f"x1f8_{sub}")
                pb = x1pool.tile([128, D], F32, tag=f"x1pb_{sub}",
                                 name=f"x1pb_{sub}")
                if apply_affine:
                    t = lnpool.tile([128, D], F32, tag="x1t", bufs=2)
                    nc.vector.tensor_scalar(t[:], y1s[sub][:], mv[:, 0:1],
                                            rstd[:, 0:1], ALU.subtract,
                                            ALU.mult)
                    nc.vector.scalar_tensor_tensor(
                        pb[:], t[:], 1.0, g1_sb[:], ALU.mult, ALU.mult)
                    nc.gpsimd.tensor_add(f8[:], pb[:], be1_sb[:])
                    nc.vector.tensor_add(pb[:], pb[:], be1_sb[:])
                    nc.vector.tensor_add(pb[:], pb[:], b2_sb[:])
                else:
                    if sub % 2 == 0:
                        nc.scalar.activation(f8[:], y1s[sub][:],
                                             AF.Identity,
                                             bias=nmr[:, 0:1],
                                             scale=rstd[:, 0:1])
                    else:
                        nc.vector.tensor_scalar(f8[:], y1s[sub][:],
                                                mv[:, 0:1], rstd[:, 0:1],
                                                ALU.subtract, ALU.mult)
                    nc.gpsimd.tensor_scalar(pb[:], y1s[sub][:], mv[:, 0:1],
                                            rstd[:, 0:1], ALU.subtract,
                                            ALU.mult)
                    nc.gpsimd.tensor_add(pb[:], pb[:], b2_sb[:])
                x1f8.append(f8)
                x1pb.append(pb)

            # ---- x1^T in DR layout via PE transpose; FFN; LN2 ----
            with (
                tc.tile_pool(name="xtps", bufs=2, space="PSUM") as xtps,
                tc.tile_pool(name="ffn", bufs=1) as ffpool,
                tc.tile_pool(name="hps", bufs=3, space="PSUM") as hps,
                tc.tile_pool(name="fps", bufs=3, space="PSUM") as fps,
                tc.tile_pool(name="out", bufs=2) as opool,
            ):
                x1dr = [ffpool.tile([128, 2, SQ], F8, tag=f"x1dr{kk}",
                                    name=f"x1dr{kk}") for kk in range(4)]
                for k in range(8):
                    pt = xtps.tile([128, 512], BF16, tag="xtps", name=f"xt{k}")
                    for sub in range(4):
                        nc.tensor.transpose(
                            pt[:, sub * 128:(sub + 1) * 128],
                            x1f8[sub][:, k * 128:(k + 1) * 128], idn[:])
                    if k % 2 == 0:
                        nc.vector.tensor_copy(x1dr[k // 2][:, k % 2, :],
                                              pt[:])
                    else:
                        nc.scalar.copy(x1dr[k // 2][:, k % 2, :], pt[:])

                # FFN1: h1^T = relu(x1 @ W1*16 + b1*16), kept at 16x scale
                h1dr = [ffpool.tile([128, 2, SQ], F8, tag=f"h1dr{jj}",
                                    name=f"h1dr{jj}") for jj in range(16)]
                for j in range(32):
                    ps = hps.tile([128, 512], F32, tag="hps", name=f"h{j}")
                    for k in range(4):
                        nc.tensor.matmul(
                            ps[:], w1_sb[k][:, :, j * 128:(j + 1) * 128],
                            x1dr[k][:], start=(k == 0), stop=(k == 3),
                            perf_mode=DR)
                    if j % 2 == 0:
                        nc.vector.tensor_scalar(
                            h1dr[j // 2][:, j % 2, :], ps[:],
                            b1_sb[:, j:j + 1], 0.0, ALU.add, ALU.max)
                    else:
                        nc.scalar.activation(
                            h1dr[j // 2][:, j % 2, :], ps[:], AF.Relu,
                            bias=b1_sb[:, j:j + 1])

                # FFN2 + residual;  psum = 256*(h1 @ W2)
                y2s = []
                for tc_ in range(4):
                    y2 = y1pool.tile([128, D], F32, tag="y2",
                                     name=f"y2_{tc_}")
                    for dg in range(2):
                        ps = fps.tile([128, 512], F32, tag="fps",
                                      name=f"f{tc_}_{dg}")
                        for jj in range(16):
                            nc.tensor.matmul(
                                ps[:],
                                h1dr[jj][:, :, tc_ * 128:(tc_ + 1) * 128],
                                w2_sb[jj][:, :, dg * 512:(dg + 1) * 512],
                                start=(jj == 0), stop=(jj == 15),
                                perf_mode=DR)
                        nc.vector.scalar_tensor_tensor(
                            y2[:, dg * 512:(dg + 1) * 512], ps[:],
                            1.0 / (WS * WS),
                            x1pb[tc_][:, dg * 512:(dg + 1) * 512],
                            ALU.mult, ALU.add)
                    y2s.append(y2)

                # LN2 -> out
                for sub in range(4):
                    mv, rstd, nmr = _ln_mv(nc, lnpool, y2s[sub], eps_sb)
                    x2 = opool.tile([128, D], F32, tag="x2")
                    if apply_affine:
                        nc.vector.tensor_scalar(
                            x2[:], y2s[sub][:], mv[:, 0:1], rstd[:, 0:1],
                            ALU.subtract, ALU.mult)
                        nc.vector.scalar_tensor_tensor(
                            x2[:], x2[:], 1.0, g2_sb[:], ALU.mult, ALU.mult)
                        nc.vector.tensor_add(x2[:], x2[:], be2_sb[:])
                    else:
                        nc.scalar.activation(x2[:], y2s[sub][:],
                                             AF.Identity,
                                             bias=nmr[:, 0:1],
                                             scale=rstd[:, 0:1])
                    nc.sync.dma_start(
                        out_q[sub * 128:(sub + 1) * 128, :], x2[:])

    nc.compile()
    _BUILD_CACHE[apply_affine] = nc
    return nc


def _ln_mv(nc, pool, y, eps_sb):
    """bn_stats-based LN stats: returns (mv [mean|var], rstd) tiles."""
    st = pool.tile([128, 2, 6], F32, tag="ln_st")
    yv = y.rearrange("p (g f) -> p g f", g=2)
    nc.vector.bn_stats(st[:, 0, :], yv[:, 0, :])
    nc.vector.bn_stats(st[:, 1, :], yv[:, 1, :])
    mv = pool.tile([128, 2], F32, tag="ln_mv")
    nc.vector.bn_aggr(mv[:], st[:])
    std = pool.tile([128, 1], F32, tag="ln_std")
    nc.scalar.activation(std[:], mv[:, 1:2], AF.Sqrt,
                         bias=eps_sb[:, 0:1])
    rstd = pool.tile([128, 1], F32, tag="ln_rstd")
    nc.vector.reciprocal(rstd[:], std[:])
    nmr = pool.tile([128, 1], F32, tag="ln_nmr")
    nc.vector.tensor_scalar(nmr[:], mv[:, 0:1], rstd[:, 0:1], -1.0,
                            ALU.mult, ALU.mult)
    return mv, rstd, nmr


def kernel(x, Wq, bq, ln1_g, ln1_b, W1, b1, W2, b2, ln2_g, ln2_b):
    x = np.asarray(x, np.float32)
    f8 = dt.np(F8)
    trivial = (np.all(ln1_g == 1) and np.all(ln1_b == 0)
               and np.all(ln2_g == 1) and np.all(ln2_b == 0))
    nc = _build(apply_affine=not trivial)

    Wqf = np.asarray(Wq, np.float32).transpose(1, 0, 2).reshape(D, D)
    base = {
        "wq_dr": np.ascontiguousarray(
            (WS * Wqf).reshape(4, 2, 128, D).transpose(0, 2, 1, 3)
        ).astype(f8),
        "bq_r": np.ascontiguousarray(
            np.asarray(bq, np.float32).reshape(8, 128).T),
        "w1_dr": np.ascontiguousarray(
            (WS * np.asarray(W1, np.float32))
            .reshape(4, 2, 128, HID).transpose(0, 2, 1, 3)).astype(f8),
        "b1_r": np.ascontiguousarray(
            (WS * np.asarray(b1, np.float32)).reshape(32, 128).T),
        "w2_dr": np.ascontiguousarray(
            (WS * np.asarray(W2, np.float32))
            .reshape(16, 2, 128, D).transpose(0, 2, 1, 3)).astype(f8),
        "b2_bc": np.ascontiguousarray(
            np.broadcast_to(np.asarray(b2, np.float32), (128, D))),
    }
    if not trivial:
        for name, v in (("g1d", ln1_g), ("be1d", ln1_b),
                        ("g2d", ln2_g), ("be2d", ln2_b)):
            base[name] = np.ascontiguousarray(
                np.broadcast_to(np.asarray(v, np.float32), (128, D)))

    in_maps = []
    for c in range(NCORES):
        b, t = divmod(c, 4)
        xb = np.concatenate([x[b, t * SQ:], x[b, :t * SQ]], axis=0)
        in_maps.append({
            **base,
            "xT_dr": np.ascontiguousarray(
                xb.T.reshape(4, 2, 128, S).transpose(0, 2, 1, 3)
            ).astype(f8),
            "x_q": np.ascontiguousarray(xb[:SQ]),
        })

    import os
    trace = bool(int(os.environ.get("KERNEL_TRACE", "0")))
    kw = {}
    if trace:
        kw = dict(trace=True,
                  tmpdir=os.environ.get("KERNEL_TRACE_DIR") or None)
    res = run_bass_kernel_spmd(nc, in_maps, core_ids=list(range(NCORES)),
                               **kw)
    out = np.empty((B, S, D), np.float32)
    for c in range(NCORES):
        b, t = divmod(c, 4)
        out[b, t * SQ:(t + 1) * SQ] = res.results[c]["out_q"]
    return out


# revision 3
# speedup vs baseline: 1.0034x; 1.0034x over previous
"""Trainium2 Bass kernel for a dense transformer encoder layer (v2).

Reference semantics (B=2, S=2048, D=1024, H=16, DH=64, HID=4096):
    q = einsum('bsd,hde->bhse', x, Wq) + bq          (q == k == v, source bug)
    prob = softmax(q @ q^T / sqrt(DH))
    attn = concat_heads(prob @ q)
    x1 = LN(x + attn);  ff = relu(x1 @ W1 + b1) @ W2 + b2;  out = LN(x1 + ff)

Sharding: 8 cores, core c -> batch b=c//4, token quarter t=c%4.  Each core
computes q for the full (rotated) sequence of its batch, then attention +
FFN for its own 512 tokens.  Zero collectives; quarters reassembled on host.

Structure:
  - Host supplies x^T pre-packed in fp8 DoubleRow layout; q projection and
    both FFN matmuls run fp8e4 DoubleRow (weights pre-scaled x16, unscaled
    in the psum epilogues; h1 is kept at 16x so relu needs no extra op).
  - q-natural layout (wv moving operand) comes from on-chip PE transposes
    of qT; no DMA transposes or DRAM round trips anywhere.
  - wv: stationary = exp(scores) [keys x queries] chunk, moving = q-natural
    [keys x 64] plus a ones column for the softmax denominator, so head
    outputs land directly in [queries, dims] layout; epilogue is one
    reciprocal + scalar_tensor_tensor into y1 per (h, qc).
  - exp(scores/8 - 1) keeps E inside fp8e4 range; softmax cancels e^-1.
  - The ACT engine (exp) is the attention bottleneck; scores pace it via
    psum rotation while wv/qproj/transpose chunks fill PE between score
    matmuls.  DMAs are scheduled so xT/Wq land first and W1/W2 stream in
    mid-attention.  LayerNorm stats use bn_stats/bn_aggr on DVE.
"""

import numpy as np

import concourse.bacc as bacc
import concourse.mybir as mybir
from concourse import tile
from concourse.bass_utils import run_bass_kernel_spmd

dt = mybir.dt
AF = mybir.ActivationFunctionType
ALU = mybir.AluOpType
DR = mybir.MatmulPerfMode.DoubleRow

B, S, D = 2, 2048, 1024
H, DH, HID = 16, 64, 4096
SQ = S // 4
NCORES = 8
EPS = 1e-5
WS = 16.0          # fp8 weight pre-scale
F32, BF16, F8 = dt.float32, dt.bfloat16, dt.float8e4

_BUILD_CACHE = {}


def _build(apply_affine: bool):
    if apply_affine in _BUILD_CACHE:
        return _BUILD_CACHE[apply_affine]

    nc = bacc.Bacc("TRN2", target_bir_lowering=False, debug=False,
                   num_devices=NCORES)

    xT_dr = nc.dram_tensor("xT_dr", [4, 128, 2, S], F8,
                           kind="ExternalInput").ap()
    x_q = nc.dram_tensor("x_q", [SQ, D], F32, kind="ExternalInput").ap()
    wq_dr = nc.dram_tensor("wq_dr", [4, 128, 2, D], F8,
                           kind="ExternalInput").ap()
    bq_r = nc.dram_tensor("bq_r", [128, 8], F32, kind="ExternalInput").ap()
    w1_dr = nc.dram_tensor("w1_dr", [4, 128, 2, HID], F8,
                           kind="ExternalInput").ap()
    b1_r = nc.dram_tensor("b1_r", [128, 32], F32, kind="ExternalInput").ap()
    w2_dr = nc.dram_tensor("w2_dr", [16, 128, 2, D], F8,
                           kind="ExternalInput").ap()
    b2_bc = nc.dram_tensor("b2_bc", [128, D], F32, kind="ExternalInput").ap()
    if apply_affine:
        g1d = nc.dram_tensor("g1d", [128, D], F32, kind="ExternalInput").ap()
        be1d = nc.dram_tensor("be1d", [128, D], F32,
                              kind="ExternalInput").ap()
        g2d = nc.dram_tensor("g2d", [128, D], F32, kind="ExternalInput").ap()
        be2d = nc.dram_tensor("be2d", [128, D], F32,
                              kind="ExternalInput").ap()
    out_q = nc.dram_tensor("out_q", [SQ, D], F32, kind="ExternalOutput").ap()

    with tile.TileContext(nc) as tc:
        with (
            tc.tile_pool(name="const", bufs=1) as cpool,
            tc.tile_pool(name="wts", bufs=1) as wpool,
            tc.tile_pool(name="y1", bufs=4) as y1pool,
            tc.tile_pool(name="x1", bufs=1) as x1pool,
            tc.tile_pool(name="ln", bufs=2) as lnpool,
        ):
            # ---- small constants (cheap DMAs first, they are tiny) ----
            bq_sb = cpool.tile([128, 8], F32)
            nc.sync.dma_start(bq_sb[:], bq_r[:])
            b1_sb = cpool.tile([128, 32], F32)
            nc.sync.dma_start(b1_sb[:], b1_r[:])

            eps_sb = cpool.tile([128, 1], F32)
            nc.vector.memset(eps_sb[:], EPS)
            neg1_sb = cpool.tile([128, 1], F32)
            nc.vector.memset(neg1_sb[:], -1.0)
            ones_f8 = cpool.tile([128, 1], F8)
            nc.vector.memset(ones_f8[:], 1.0)

            # fp8 identity for PE-mode transposes
            col_i = cpool.tile([128, 128], F32)
            nc.gpsimd.iota(col_i[:], [[1, 128]], channel_multiplier=0,
                           allow_small_or_imprecise_dtypes=True)
            row_i = cpool.tile([128, 1], F32)
            nc.gpsimd.iota(row_i[:], [[0, 1]], channel_multiplier=1,
                           allow_small_or_imprecise_dtypes=True)
            idn = cpool.tile([128, 128], BF16)
            nc.vector.tensor_scalar(idn[:], col_i[:], row_i[:, 0:1], None,
                                    ALU.is_equal)

            b2_sb = cpool.tile([128, D], F32)
            y1s = [y1pool.tile([128, D], F32, tag="y1", name=f"y1_{sub}")
                   for sub in range(4)]
            w1_sb = [wpool.tile([128, 2, HID], F8, tag=f"w1{k}",
                                name=f"w1_{k}") for k in range(4)]
            w2_sb = [wpool.tile([128, 2, D], F8, tag=f"w2{j}",
                                name=f"w2_{j}") for j in range(16)]
            if apply_affine:
                g1_sb = cpool.tile([128, D], F32)
                nc.sync.dma_start(g1_sb[:], g1d[:])
                be1_sb = cpool.tile([128, D], F32)
                nc.sync.dma_start(be1_sb[:], be1d[:])
                g2_sb = cpool.tile([128, D], F32)
                nc.sync.dma_start(g2_sb[:], g2d[:])
                be2_sb = cpool.tile([128, D], F32)
                nc.sync.dma_start(be2_sb[:], be2d[:])

            with (
                tc.tile_pool(name="qT", bufs=1) as qTpool,
                tc.tile_pool(name="qa", bufs=12) as qapool,
                tc.tile_pool(name="E", bufs=18) as Epool,
                tc.tile_pool(name="xw", bufs=1) as xwpool,
                tc.tile_pool(name="qps", bufs=1, space="PSUM") as qps,
                tc.tile_pool(name="scps", bufs=2, space="PSUM") as scps,
                tc.tile_pool(name="tps", bufs=1, space="PSUM") as tps,
                tc.tile_pool(name="uvps", bufs=2, space="PSUM") as uvps,
            ):
                # xT / Wq first: they gate the whole pipeline
                wq_sb = []
                xT_sb = []
                for k in range(4):
                    t = xwpool.tile([128, 2, D], F8, tag=f"wq{k}",
                                    name=f"wq_{k}")
                    nc.sync.dma_start(t[:, :, 0:128], wq_dr[k][:, :, 0:128])
                    wq_sb.append(t)
                    t = xwpool.tile([128, 2, S], F8, tag=f"xT{k}",
                                    name=f"xT_{k}")
                    nc.sync.dma_start(t[:, :, 0:512], xT_dr[k][:, :, 0:512])
                    xT_sb.append(t)
                for k in range(4):
                    nc.sync.dma_start(xT_sb[k][:, :, 512:S],
                                      xT_dr[k][:, :, 512:S])
                for k in range(4):
                    nc.sync.dma_start(wq_sb[k][:, :, 128:D],
                                      wq_dr[k][:, :, 128:D])
                qT = [qTpool.tile([128, S], BF16, tag=f"qT{p}",
                                  name=f"qT{p}") for p in range(8)]
                qa = [[None] * 4 for _ in range(8)]

                def emit_qproj(p):
                    """qT[p] = (x @ Wq[:, pair p])^T via DR matmuls + bias."""
                    for n in range(4):
                        ps = qps.tile([128, 512], F32, tag="qps",
                                      name=f"qps{p}_{n}")
                        for k in range(4):
                            nc.tensor.matmul(
                                ps[:],
                                wq_sb[k][:, :, p * 128:(p + 1) * 128],
                                xT_sb[k][:, :, n * 512:(n + 1) * 512],
                                start=(k == 0), stop=(k == 3), perf_mode=DR)
                        nc.vector.tensor_scalar(
                            qT[p][:, n * 512:(n + 1) * 512], ps[:],
                            1.0 / WS, bq_sb[:, p:p + 1], ALU.mult, ALU.add)

                def emit_qtrans(p):
                    """qa[p][g]: 4 chunks x [one|h0 d64|one|h1 d64] via PE
                    transpose.  The leading ones give the softmax
                    denominator as row 0 of the wv output."""
                    for g in range(4):
                        pt = tps.tile([128, 512], BF16, tag="tps",
                                      name=f"tps{p}_{g}")
                        for j in range(4):
                            c = 4 * g + j
                            nc.tensor.transpose(
                                pt[:, j * 128:(j + 1) * 128],
                                qT[p][:, c * 128:(c + 1) * 128], idn[:])
                        t = qapool.tile([128, 4, 2, 65], F8, tag="qa",
                                        name=f"qa{p}_{g}")
                        nc.gpsimd.memset(t[:, :, :, 0:1], 1.0)
                        nc.vector.tensor_copy(
                            t[:, :, :, 1:65],
                            pt[:].rearrange("p (c h d) -> p c h d",
                                            c=4, h=2))
                        qa[p][g] = t

                E = {}      # (h, kp) -> fp8 [128, 1024] tile

                def emit_scores_kp(h, kp):
                    p, half = h // 2, h % 2
                    rows = slice(half * 64, half * 64 + 64)
                    sc = scps.tile([128, 1024], F32, tag="sc",
                                   name=f"sc{h}_{kp}")
                    for kk in range(2):
                        c = 2 * kp + kk
                        nc.tensor.matmul(
                            sc[:, kk * 512:(kk + 1) * 512],
                            qT[p][rows, c * 128:(c + 1) * 128],
                            qT[p][rows, 0:512],
                            start=True, stop=True)
                    e = Epool.tile([128, 1024], F8, tag="E",
                                   name=f"E{h}_{kp}")
                    nc.scalar.activation(e[:], sc[:], AF.Exp, scale=0.125,
                                         bias=neg1_sb[:, 0:1])
                    E[(h, kp)] = e

                def emit_wv_qc(h, qc):
                    p, half = h // 2, h % 2
                    uv = uvps.tile([128, 65], F32, tag="uv",
                                   name=f"uv{h}_{qc}")
                    for kc in range(16):
                        est = E[(h, kc // 2)][
                            :, (kc % 2) * 512 + qc * 128:
                               (kc % 2) * 512 + (qc + 1) * 128]
                        qam = qa[p][kc // 4][:, kc % 4, half, :]
                        nc.tensor.matmul(uv[:], est, qam,
                                         start=(kc == 0), stop=(kc == 15))
                    rct = lnpool.tile([128, 1], F32, tag="rct", bufs=4,
                                      name=f"rct{h}_{qc}")
                    nc.vector.reciprocal(rct[:], uv[:, 0:1])
                    nc.vector.scalar_tensor_tensor(
                        y1s[qc][:, h * 64:(h + 1) * 64],
                        uv[:, 1:65], rct[:, 0:1],
                        y1s[qc][:, h * 64:(h + 1) * 64],
                        ALU.mult, ALU.add)

                # ---- attention pipeline, ACT(exp)-paced ----
                emit_qproj(0)
                emit_qtrans(0)
                # residual rows + FFN consts: after xT/wq on the DMA queue
                for sub in range(4):
                    nc.sync.dma_start(y1s[sub][:],
                                      x_q[sub * 128:(sub + 1) * 128, :])
                nc.sync.dma_start(b2_sb[:], b2_bc[:])

                for h in range(16):
                    nxt = h // 2 + 1
                    for kp in range(8):
                        emit_scores_kp(h, kp)
                        if h >= 1 and 1 <= kp <= 4:
                            emit_wv_qc(h - 1, kp - 1)
                        if h % 2 == 0 and nxt < 8:
                            if kp == 5:
                                emit_qproj(nxt)
                            elif kp == 6:
                                emit_qtrans(nxt)
                    if h == 5:
                        for k in range(4):
                            nc.sync.dma_start(w1_sb[k][:], w1_dr[k])
                    if h == 9:
                        for j in range(16):
                            nc.sync.dma_start(w2_sb[j][:], w2_dr[j])
                for qc in range(4):
                    emit_wv_qc(15, qc)

            # ---- LN1 -> x1 (fp8 + f32+b2 variants) ----
            x1f8 = []
            x1pb = []
            for sub in range(4):
                mv, rstd, nmr = _ln_mv(nc, lnpool, y1s[sub], eps_sb)
                f8 = x1pool.tile([128, D], BF16, tag=f"x1f8_{sub}",
                                 name=f"x1f8_{sub}")
                pb = x1pool.tile([128, D], F32, tag=f"x1pb_{sub}",
                                 name=f"x1pb_{sub}")
                if apply_affine:
                    t = lnpool.tile([128, D], F32, tag="x1t", bufs=2)
                    nc.vector.tensor_scalar(t[:], y1s[sub][:], mv[:, 0:1],
                                            rstd[:, 0:1], ALU.subtract,
                                            ALU.mult)
                    nc.vector.scalar_tensor_tensor(
                        pb[:], t[:], 1.0, g1_sb[:], ALU.mult, ALU.mult)
                    nc.gpsimd.tensor_add(f8[:], pb[:], be1_sb[:])
                    nc.vector.tensor_add(pb[:], pb[:], be1_sb[:])
                    nc.vector.tensor_add(pb[:], pb[:], b2_sb[:])
                else:
                    if sub % 2 == 0:
                        nc.scalar.activation(f8[:], y1s[sub][:],
                                             AF.Identity,
                                             bias=nmr[:, 0:1],
                                             scale=rstd[:, 0:1])
                    else:
                        nc.vector.tensor_scalar(f8[:], y1s[sub][:],
                                                mv[:, 0:1], rstd[:, 0:1],
                                                ALU.subtract, ALU.mult)
                    nc.gpsimd.tensor_scalar(pb[:], y1s[sub][:], mv[:, 0:1],
                                            rstd[:, 0:1], ALU.subtract,
                                            ALU.mult)
                    nc.gpsimd.tensor_add(pb[:], pb[:], b2_sb[:])
                x1f8.append(f8)
                x1pb.append(pb)

            # ---- x1^T in DR layout via PE transpose; FFN; LN2 ----
            with (
                tc.tile_pool(name="ffn", bufs=1) as ffpool,
                tc.tile_pool(name="hps", bufs=4, space="PSUM") as hps,
                tc.tile_pool(name="fps", bufs=4, space="PSUM") as fps,
                tc.tile_pool(name="out", bufs=2) as opool,
            ):
                x1dr = [ffpool.tile([128, 2, SQ], F8, tag=f"x1dr{kk}",
                                    name=f"x1dr{kk}") for kk in range(4)]
                for kk in range(4):
                    ptf = hps.tile([128, 512], F32, tag="hps",
                                   name=f"xt{kk}")
                    pt = ptf[:].bitcast(BF16)
                    for k2 in range(2):
                        k = 2 * kk + k2
                        for sub in range(4):
                            nc.tensor.transpose(
                                pt[:, k2 * 512 + sub * 128:
                                   k2 * 512 + (sub + 1) * 128],
                                x1f8[sub][:, k * 128:(k + 1) * 128],
                                idn[:])
                    for k2 in range(2):
                        sl = pt[:, k2 * 512:(k2 + 1) * 512]
                        if k2 == 0:
                            nc.vector.tensor_copy(
                                x1dr[kk][:, 0, :], sl)
                        else:
                            nc.scalar.copy(x1dr[kk][:, 1, :], sl)

                # FFN1: h1^T = relu(x1 @ W1*16 + b1*16), kept at 16x scale
                h1dr = [ffpool.tile([128, 2, SQ], F8, tag=f"h1dr{jj}",
                                    name=f"h1dr{jj}") for jj in range(16)]
                for j in range(32):
                    ps = hps.tile([128, 512], F32, tag="hps", name=f"h{j}")
                    for k in range(4):
                        nc.tensor.matmul(
                            ps[:], w1_sb[k][:, :, j * 128:(j + 1) * 128],
                            x1dr[k][:], start=(k == 0), stop=(k == 3),
                            perf_mode=DR)
                    if j % 2 == 0:
                        nc.vector.tensor_scalar(
                            h1dr[j // 2][:, j % 2, :], ps[:],
                            b1_sb[:, j:j + 1], 0.0, ALU.add, ALU.max)
                    else:
                        nc.scalar.activation(
                            h1dr[j // 2][:, j % 2, :], ps[:], AF.Relu,
                            bias=b1_sb[:, j:j + 1])

                # FFN2 + residual;  psum = 256*(h1 @ W2)
                y2s = []
                for tc_ in range(4):
                    y2 = y1pool.tile([128, D], F32, tag="y2",
                                     name=f"y2_{tc_}")
                    for dg in range(2):
                        ps = fps.tile([128, 512], F32, tag="fps",
                                      name=f"f{tc_}_{dg}")
                        for jj in range(16):
                            nc.tensor.matmul(
                                ps[:],
                                h1dr[jj][:, :, tc_ * 128:(tc_ + 1) * 128],
                                w2_sb[jj][:, :, dg * 512:(dg + 1) * 512],
                                start=(jj == 0), stop=(jj == 15),
                                perf_mode=DR)
                        nc.vector.scalar_tensor_tensor(
                            y2[:, dg * 512:(dg + 1) * 512], ps[:],
                            1.0 / (WS * WS),
                            x1pb[tc_][:, dg * 512:(dg + 1) * 512],
                            ALU.mult, ALU.add)
                    y2s.append(y2)

                # LN2 -> out
                for sub in range(4):
                    mv, rstd, nmr = _ln_mv(nc, lnpool, y2s[sub], eps_sb)
                    x2 = opool.tile([128, D], F32, tag="x2")
                    if apply_affine:
                        nc.vector.tensor_scalar(
                            x2[:], y2s[sub][:], mv[:, 0:1], rstd[:, 0:1],
                            ALU.subtract, ALU.mult)
                        nc.vector.scalar_tensor_tensor(
                            x2[:], x2[:], 1.0, g2_sb[:], ALU.mult, ALU.mult)
                        nc.vector.tensor_add(x2[:], x2[:], be2_sb[:])
                    else:
                        nc.scalar.activation(x2[:], y2s[sub][:],
                                             AF.Identity,
                                             bias=nmr[:, 0:1],
                                             scale=rstd[:, 0:1])
                    nc.sync.dma_start(
                        out_q[sub * 128:(sub + 1) * 128, :], x2[:])

    nc.compile()
    _BUILD_CACHE[apply_affine] = nc
    return nc


def _ln_mv(nc, pool, y, eps_sb):
    """bn_stats-based LN stats: returns (mv [mean|var], rstd) tiles."""
    st = pool.tile([128, 2, 6], F32, tag="ln_st")
    yv = y.rearrange("p (g f) -> p g f", g=2)
    nc.vector.bn_stats(st[:, 0, :], yv[:, 0, :])
    nc.vector.bn_stats(st[:, 1, :], yv[:, 1, :])
    mv = pool.tile([128, 2], F32, tag="ln_mv")
    nc.vector.bn_aggr(mv[:], st[:])
    std = pool.tile([128, 1], F32, tag="ln_std")
    nc.scalar.activation(std[:], mv[:, 1:2], AF.Sqrt,
                         bias=eps_sb[:, 0:1])
    rstd = pool.tile([128, 1], F32, tag="ln_rstd")
    nc.vector.reciprocal(rstd[:], std[:])
    nmr = pool.tile([128, 1], F32, tag="ln_nmr")
    nc.vector.tensor_scalar(nmr[:], mv[:, 0:1], rstd[:, 0:1], -1.0,
                            ALU.mult, ALU.mult)
    return mv, rstd, nmr


def kernel(x, Wq, bq, ln1_g, ln1_b, W1, b1, W2, b2, ln2_g, ln2_b):
    x = np.asarray(x, np.float32)
    f8 = dt.np(F8)
    trivial = (np.all(ln1_g == 1) and np.all(ln1_b == 0)
               and np.all(ln2_g == 1) and np.all(ln2_b == 0))
    nc = _build(apply_affine=not trivial)

    Wqf = np.asarray(Wq, np.float32).transpose(1, 0, 2).reshape(D, D)
    base = {
        "wq_dr": np.ascontiguousarray(
            (WS * Wqf).reshape(4, 2, 128, D).transpose(0, 2, 1, 3)
        ).astype(f8),
        "bq_r": np.ascontiguousarray(
            np.asarray(bq, np.float32).reshape(8, 128).T),
        "w1_dr": np.ascontiguousarray(
            (WS * np.asarray(W1, np.float32))
            .reshape(4, 2, 128, HID).transpose(0, 2, 1, 3)).astype(f8),
        "b1_r": np.ascontiguousarray(
            (WS * np.asarray(b1, np.float32)).reshape(32, 128).T),
        "w2_dr": np.ascontiguousarray(
            (WS * np.asarray(W2, np.float32))
            .reshape(16, 2, 128, D).transpose(0, 2, 1, 3)).astype(f8),
        "b2_bc": np.ascontiguousarray(
            np.broadcast_to(np.asarray(b2, np.float32), (128, D))),
    }
    if not trivial:
        for name, v in (("g1d", ln1_g), ("be1d", ln1_b),
                        ("g2d", ln2_g), ("be2d", ln2_b)):
            base[name] = np.ascontiguousarray(
                np.broadcast_to(np.asarray(v, np.float32), (128, D)))

    in_maps = []
    for c in range(NCORES):
        b, t = divmod(c, 4)
        xb = np.concatenate([x[b, t * SQ:], x[b, :t * SQ]], axis=0)
        in_maps.append({
            **base,
            "xT_dr": np.ascontiguousarray(
                xb.T.reshape(4, 2, 128, S).transpose(0, 2, 1, 3)
            ).astype(f8),
            "x_q": np.ascontiguousarray(xb[:SQ]),
        })

    import os
    trace = bool(int(os.environ.get("KERNEL_TRACE", "0")))
    kw = {}
    if trace:
        kw = dict(trace=True,
                  tmpdir=os.environ.get("KERNEL_TRACE_DIR") or None)
    res = run_bass_kernel_spmd(nc, in_maps, core_ids=list(range(NCORES)),
                               **kw)
    out = np.empty((B, S, D), np.float32)
    for c in range(NCORES):
        b, t = divmod(c, 4)
        out[b, t * SQ:(t + 1) * SQ] = res.results[c]["out_q"]
    return out


# revision 4
# speedup vs baseline: 1.0112x; 1.0078x over previous
"""Trainium2 Bass kernel for a dense transformer encoder layer (v2).

Reference semantics (B=2, S=2048, D=1024, H=16, DH=64, HID=4096):
    q = einsum('bsd,hde->bhse', x, Wq) + bq          (q == k == v, source bug)
    prob = softmax(q @ q^T / sqrt(DH))
    attn = concat_heads(prob @ q)
    x1 = LN(x + attn);  ff = relu(x1 @ W1 + b1) @ W2 + b2;  out = LN(x1 + ff)

Sharding: 8 cores, core c -> batch b=c//4, token quarter t=c%4.  Each core
computes q for the full (rotated) sequence of its batch, then attention +
FFN for its own 512 tokens.  Zero collectives; quarters reassembled on host.

Structure:
  - Host supplies x^T pre-packed in fp8 DoubleRow layout; q projection and
    both FFN matmuls run fp8e4 DoubleRow (weights pre-scaled x16, unscaled
    in the psum epilogues; h1 is kept at 16x so relu needs no extra op).
  - q-natural layout (wv moving operand) comes from on-chip PE transposes
    of qT; no DMA transposes or DRAM round trips anywhere.
  - wv: stationary = exp(scores) [keys x queries] chunk, moving = q-natural
    [keys x 64] plus a ones column for the softmax denominator, so head
    outputs land directly in [queries, dims] layout; epilogue is one
    reciprocal + scalar_tensor_tensor into y1 per (h, qc).
  - exp(scores/8 - 1) keeps E inside fp8e4 range; softmax cancels e^-1.
  - The ACT engine (exp) is the attention bottleneck; scores pace it via
    psum rotation while wv/qproj/transpose chunks fill PE between score
    matmuls.  DMAs are scheduled so xT/Wq land first and W1/W2 stream in
    mid-attention.  LayerNorm stats use bn_stats/bn_aggr on DVE.
"""

import numpy as np

import concourse.bacc as bacc
import concourse.mybir as mybir
from concourse import tile
from concourse.bass_utils import run_bass_kernel_spmd

dt = mybir.dt
AF = mybir.ActivationFunctionType
ALU = mybir.AluOpType
DR = mybir.MatmulPerfMode.DoubleRow

B, S, D = 2, 2048, 1024
H, DH, HID = 16, 64, 4096
SQ = S // 4
NCORES = 8
EPS = 1e-5
WS = 16.0          # fp8 weight pre-scale
F32, BF16, F8 = dt.float32, dt.bfloat16, dt.float8e4

_BUILD_CACHE = {}


def _build(apply_affine: bool):
    if apply_affine in _BUILD_CACHE:
        return _BUILD_CACHE[apply_affine]

    nc = bacc.Bacc("TRN2", target_bir_lowering=False, debug=False,
                   num_devices=NCORES)

    xT_dr = nc.dram_tensor("xT_dr", [4, 128, 2, S], F8,
                           kind="ExternalInput").ap()
    x_q = nc.dram_tensor("x_q", [SQ, D], F32, kind="ExternalInput").ap()
    wq_dr = nc.dram_tensor("wq_dr", [4, 128, 2, D], F8,
                           kind="ExternalInput").ap()
    bq_r = nc.dram_tensor("bq_r", [128, 8], F32, kind="ExternalInput").ap()
    w1_dr = nc.dram_tensor("w1_dr", [4, 128, 2, HID], F8,
                           kind="ExternalInput").ap()
    c1_d = nc.dram_tensor("c1_d", [1, 2, HID], F8,
                          kind="ExternalInput").ap()
    w2_dr = nc.dram_tensor("w2_dr", [16, 128, 2, D], F8,
                           kind="ExternalInput").ap()
    b2_bc = nc.dram_tensor("b2_bc", [128, D], F32, kind="ExternalInput").ap()
    if apply_affine:
        g1d = nc.dram_tensor("g1d", [128, D], F32, kind="ExternalInput").ap()
        be1d = nc.dram_tensor("be1d", [128, D], F32,
                              kind="ExternalInput").ap()
        g2d = nc.dram_tensor("g2d", [128, D], F32, kind="ExternalInput").ap()
        be2d = nc.dram_tensor("be2d", [128, D], F32,
                              kind="ExternalInput").ap()
    out_q = nc.dram_tensor("out_q", [SQ, D], F32, kind="ExternalOutput").ap()

    with tile.TileContext(nc) as tc:
        with (
            tc.tile_pool(name="const", bufs=1) as cpool,
            tc.tile_pool(name="wts", bufs=1) as wpool,
            tc.tile_pool(name="y1", bufs=4) as y1pool,
            tc.tile_pool(name="x1", bufs=1) as x1pool,
            tc.tile_pool(name="ln", bufs=2) as lnpool,
        ):
            # ---- small constants (cheap DMAs first, they are tiny) ----
            bq_sb = cpool.tile([128, 8], F32)
            nc.sync.dma_start(bq_sb[:], bq_r[:])
            c1_sb = cpool.tile([1, 2, HID], F8)
            nc.sync.dma_start(c1_sb[:], c1_d[:])

            eps_sb = cpool.tile([128, 1], F32)
            nc.vector.memset(eps_sb[:], EPS)
            neg1_sb = cpool.tile([128, 1], F32)
            nc.vector.memset(neg1_sb[:], -1.0)
            ones_f8 = cpool.tile([128, 1], F8)
            nc.vector.memset(ones_f8[:], 1.0)

            # fp8 identity for PE-mode transposes
            col_i = cpool.tile([128, 128], F32)
            nc.gpsimd.iota(col_i[:], [[1, 128]], channel_multiplier=0,
                           allow_small_or_imprecise_dtypes=True)
            row_i = cpool.tile([128, 1], F32)
            nc.gpsimd.iota(row_i[:], [[0, 1]], channel_multiplier=1,
                           allow_small_or_imprecise_dtypes=True)
            idn = cpool.tile([128, 128], BF16)
            nc.vector.tensor_scalar(idn[:], col_i[:], row_i[:, 0:1], None,
                                    ALU.is_equal)

            b2_sb = cpool.tile([128, D], F32)
            y1s = [y1pool.tile([128, D], F32, tag="y1", name=f"y1_{sub}")
                   for sub in range(4)]
            w1_sb = [wpool.tile([128, 2, HID], F8, tag=f"w1{k}",
                                name=f"w1_{k}") for k in range(4)]
            w2_sb = [wpool.tile([128, 2, D], F8, tag=f"w2{j}",
                                name=f"w2_{j}") for j in range(16)]
            if apply_affine:
                g1_sb = cpool.tile([128, D], F32)
                nc.sync.dma_start(g1_sb[:], g1d[:])
                be1_sb = cpool.tile([128, D], F32)
                nc.sync.dma_start(be1_sb[:], be1d[:])
                g2_sb = cpool.tile([128, D], F32)
                nc.sync.dma_start(g2_sb[:], g2d[:])
                be2_sb = cpool.tile([128, D], F32)
                nc.sync.dma_start(be2_sb[:], be2d[:])

            with (
                tc.tile_pool(name="qT", bufs=1) as qTpool,
                tc.tile_pool(name="qa", bufs=12) as qapool,
                tc.tile_pool(name="E", bufs=18) as Epool,
                tc.tile_pool(name="xw", bufs=1) as xwpool,
                tc.tile_pool(name="qps", bufs=1, space="PSUM") as qps,
                tc.tile_pool(name="scps", bufs=2, space="PSUM") as scps,
                tc.tile_pool(name="tps", bufs=1, space="PSUM") as tps,
                tc.tile_pool(name="uvps", bufs=2, space="PSUM") as uvps,
            ):
                # xT / Wq first: they gate the whole pipeline
                wq_sb = []
                xT_sb = []
                for k in range(4):
                    t = xwpool.tile([128, 2, D], F8, tag=f"wq{k}",
                                    name=f"wq_{k}")
                    nc.sync.dma_start(t[:, :, 0:128], wq_dr[k][:, :, 0:128])
                    wq_sb.append(t)
                    t = xwpool.tile([128, 2, S], F8, tag=f"xT{k}",
                                    name=f"xT_{k}")
                    nc.sync.dma_start(t[:, :, 0:512], xT_dr[k][:, :, 0:512])
                    xT_sb.append(t)
                for k in range(4):
                    nc.sync.dma_start(xT_sb[k][:, :, 512:S],
                                      xT_dr[k][:, :, 512:S])
                for k in range(4):
                    nc.sync.dma_start(wq_sb[k][:, :, 128:D],
                                      wq_dr[k][:, :, 128:D])
                qT = [qTpool.tile([128, S], BF16, tag=f"qT{p}",
                                  name=f"qT{p}") for p in range(8)]
                qa = [[None] * 4 for _ in range(8)]

                def emit_qproj(p):
                    """qT[p] = (x @ Wq[:, pair p])^T via DR matmuls + bias."""
                    for n in range(4):
                        ps = qps.tile([128, 512], F32, tag="qps",
                                      name=f"qps{p}_{n}")
                        for k in range(4):
                            nc.tensor.matmul(
                                ps[:],
                                wq_sb[k][:, :, p * 128:(p + 1) * 128],
                                xT_sb[k][:, :, n * 512:(n + 1) * 512],
                                start=(k == 0), stop=(k == 3), perf_mode=DR)
                        nc.vector.tensor_scalar(
                            qT[p][:, n * 512:(n + 1) * 512], ps[:],
                            1.0 / WS, bq_sb[:, p:p + 1], ALU.mult, ALU.add)

                def emit_qtrans(p):
                    """qa[p][g]: 4 chunks x [one|h0 d64|one|h1 d64] via PE
                    transpose.  The leading ones give the softmax
                    denominator as row 0 of the wv output."""
                    for g in range(4):
                        pt = tps.tile([128, 512], BF16, tag="tps",
                                      name=f"tps{p}_{g}")
                        for j in range(4):
                            c = 4 * g + j
                            nc.tensor.transpose(
                                pt[:, j * 128:(j + 1) * 128],
                                qT[p][:, c * 128:(c + 1) * 128], idn[:])
                        t = qapool.tile([128, 4, 2, 65], F8, tag="qa",
                                        name=f"qa{p}_{g}")
                        nc.gpsimd.memset(t[:, :, :, 0:1], 1.0)
                        nc.vector.tensor_copy(
                            t[:, :, :, 1:65],
                            pt[:].rearrange("p (c h d) -> p c h d",
                                            c=4, h=2))
                        qa[p][g] = t

                E = {}      # (h, kp) -> fp8 [128, 1024] tile

                def emit_scores_kp(h, kp):
                    p, half = h // 2, h % 2
                    rows = slice(half * 64, half * 64 + 64)
                    sc = scps.tile([128, 1024], F32, tag="sc",
                                   name=f"sc{h}_{kp}")
                    for kk in range(2):
                        c = 2 * kp + kk
                        nc.tensor.matmul(
                            sc[:, kk * 512:(kk + 1) * 512],
                            qT[p][rows, c * 128:(c + 1) * 128],
                            qT[p][rows, 0:512],
                            start=True, stop=True)
                    e = Epool.tile([128, 1024], F8, tag="E",
                                   name=f"E{h}_{kp}")
                    nc.scalar.activation(e[:], sc[:], AF.Exp, scale=0.125,
                                         bias=neg1_sb[:, 0:1])
                    E[(h, kp)] = e

                def emit_wv_qc(h, qc):
                    p, half = h // 2, h % 2
                    uv = uvps.tile([128, 65], F32, tag="uv",
                                   name=f"uv{h}_{qc}")
                    for kc in range(16):
                        est = E[(h, kc // 2)][
                            :, (kc % 2) * 512 + qc * 128:
                               (kc % 2) * 512 + (qc + 1) * 128]
                        qam = qa[p][kc // 4][:, kc % 4, half, :]
                        nc.tensor.matmul(uv[:], est, qam,
                                         start=(kc == 0), stop=(kc == 15))
                    rct = lnpool.tile([128, 1], F32, tag="rct", bufs=4,
                                      name=f"rct{h}_{qc}")
                    nc.vector.reciprocal(rct[:], uv[:, 0:1])
                    nc.vector.scalar_tensor_tensor(
                        y1s[qc][:, h * 64:(h + 1) * 64],
                        uv[:, 1:65], rct[:, 0:1],
                        y1s[qc][:, h * 64:(h + 1) * 64],
                        ALU.mult, ALU.add)

                # ---- attention pipeline, ACT(exp)-paced ----
                emit_qproj(0)
                emit_qtrans(0)
                # residual rows + FFN consts: after xT/wq on the DMA queue
                for sub in range(4):
                    nc.sync.dma_start(y1s[sub][:],
                                      x_q[sub * 128:(sub + 1) * 128, :])
                nc.sync.dma_start(b2_sb[:], b2_bc[:])

                for h in range(16):
                    nxt = h // 2 + 1
                    for kp in range(8):
                        emit_scores_kp(h, kp)
                        if h >= 1 and 1 <= kp <= 4:
                            emit_wv_qc(h - 1, kp - 1)
                        if h % 2 == 0 and nxt < 8:
                            if kp == 5:
                                emit_qproj(nxt)
                            elif kp == 6:
                                emit_qtrans(nxt)
                    if h == 5:
                        for k in range(4):
                            nc.sync.dma_start(w1_sb[k][:], w1_dr[k])
                    if h == 9:
                        for j in range(16):
                            nc.sync.dma_start(w2_sb[j][:], w2_dr[j])
                for qc in range(4):
                    emit_wv_qc(15, qc)

            # ---- LN1 stats only (LN affine is folded into FFN1) ----
            x1pb = []
            mss = []
            rs256 = []
            for sub in range(4):
                mv, rstd, nmr, std = _ln_mv(nc, lnpool, y1s[sub], eps_sb)
                pb = x1pool.tile([128, D], F32, tag=f"x1pb_{sub}",
                                 name=f"x1pb_{sub}")
                # ms rows for the rank-1 LN correction: [-mu | std]
                ms = lnpool.tile([128, 2], BF16, tag="ln_ms", bufs=4,
                                 name=f"ms{sub}")
                nc.vector.tensor_scalar(ms[:, 0:1], mv[:, 0:1], -WS, None,
                                        ALU.mult)
                nc.vector.tensor_copy(ms[:, 1:2], std[:, 0:1])
                rs = lnpool.tile([128, 1], F32, tag="ln_rs", bufs=4,
                                 name=f"rs{sub}")
                nc.vector.tensor_scalar(rs[:], rstd[:], 1.0 / (WS * WS),
                                        None, ALU.mult)
                mss.append(ms)
                rs256.append(rs)
                if apply_affine:
                    t = lnpool.tile([128, D], F32, tag="x1t", bufs=2)
                    nc.vector.tensor_scalar(t[:], y1s[sub][:], mv[:, 0:1],
                                            rstd[:, 0:1], ALU.subtract,
                                            ALU.mult)
                    nc.vector.scalar_tensor_tensor(
                        pb[:], t[:], 1.0, g1_sb[:], ALU.mult, ALU.mult)
                    nc.vector.tensor_add(pb[:], pb[:], be1_sb[:])
                    nc.vector.tensor_add(pb[:], pb[:], b2_sb[:])
                else:
                    nc.gpsimd.tensor_scalar(pb[:], y1s[sub][:], mv[:, 0:1],
                                            rstd[:, 0:1], ALU.subtract,
                                            ALU.mult)
                    nc.gpsimd.tensor_add(pb[:], pb[:], b2_sb[:])
                x1pb.append(pb)

            # ---- x1^T in DR layout via PE transpose; FFN; LN2 ----
            with (
                tc.tile_pool(name="ffn", bufs=1) as ffpool,
                tc.tile_pool(name="hps", bufs=4, space="PSUM") as hps,
                tc.tile_pool(name="fps", bufs=4, space="PSUM") as fps,
                tc.tile_pool(name="out", bufs=2) as opool,
            ):
                x1dr = [ffpool.tile([128, 2, SQ], F8, tag=f"x1dr{kk}",
                                    name=f"x1dr{kk}") for kk in range(4)]
                idf = ffpool.tile([128, 128], F32, tag="idf")
                nc.vector.tensor_copy(idf[:], idn[:])
                for k in range(8):
                    ptf = hps.tile([128, 512], F32, tag="hps",
                                   name=f"xt{k}")
                    for sub in range(4):
                        nc.tensor.transpose(
                            ptf[:, sub * 128:(sub + 1) * 128],
                            y1s[sub][:, k * 128:(k + 1) * 128], idf[:])
                    if k % 2 == 0:
                        nc.vector.tensor_copy(x1dr[k // 2][:, 0, :],
                                              ptf[:])
                    else:
                        nc.scalar.copy(x1dr[k // 2][:, 1, :], ptf[:])
                # [-mu | std] rows transposed to [2, 512] for the rank-1 fix
                msT = ffpool.tile([1, 2, SQ], F8, tag="msT")
                ptf = hps.tile([128, 512], F32, tag="hps", name="mst")
                ptb = ptf[:].bitcast(BF16)
                for sub in range(4):
                    nc.tensor.transpose(
                        ptb[0:1, sub * 128:(sub + 1) * 128],
                        mss[sub][:, 0:1], idn[:])
                    nc.tensor.transpose(
                        ptb[0:1, 512 + sub * 128:512 + (sub + 1) * 128],
                        mss[sub][:, 1:2], idn[:])
                nc.vector.tensor_copy(
                    msT[0:1, :, :],
                    ptb[0:1, 0:1024].rearrange("p (a f) -> p a f", a=2))

                # FFN1: h1^T = relu(x1 @ W1*16 + b1*16), kept at 16x scale
                h1dr = [ffpool.tile([128, 2, SQ], F8, tag=f"h1dr{jj}",
                                    name=f"h1dr{jj}") for jj in range(16)]
                for j in range(32):
                    ps = hps.tile([128, 512], F32, tag="hps", name=f"h{j}")
                    for k in range(4):
                        nc.tensor.matmul(
                            ps[:], w1_sb[k][:, :, j * 128:(j + 1) * 128],
                            x1dr[k][:], start=(k == 0), stop=False,
                            perf_mode=DR)
                    nc.tensor.matmul(
                        ps[:], c1_sb[:, :, j * 128:(j + 1) * 128], msT[:],
                        start=False, stop=True, perf_mode=DR)
                    if j % 2 == 0:
                        nc.vector.tensor_scalar(
                            h1dr[j // 2][:, j % 2, :], ps[:],
                            0.0, None, ALU.max)
                    else:
                        nc.scalar.activation(
                            h1dr[j // 2][:, j % 2, :], ps[:], AF.Relu)

                # FFN2 + residual;  psum = 256*(h1 @ W2)
                y2s = []
                for tc_ in range(4):
                    y2 = y1pool.tile([128, D], F32, tag="y2",
                                     name=f"y2_{tc_}")
                    for dg in range(2):
                        ps = fps.tile([128, 512], F32, tag="fps",
                                      name=f"f{tc_}_{dg}")
                        for jj in range(16):
                            nc.tensor.matmul(
                                ps[:],
                                h1dr[jj][:, :, tc_ * 128:(tc_ + 1) * 128],
                                w2_sb[jj][:, :, dg * 512:(dg + 1) * 512],
                                start=(jj == 0), stop=(jj == 15),
                                perf_mode=DR)
                        nc.vector.scalar_tensor_tensor(
                            y2[:, dg * 512:(dg + 1) * 512], ps[:],
                            rs256[tc_][:, 0:1],
                            x1pb[tc_][:, dg * 512:(dg + 1) * 512],
                            ALU.mult, ALU.add)
                    y2s.append(y2)

                # LN2 -> out
                for sub in range(4):
                    mv, rstd, nmr, std = _ln_mv(nc, lnpool, y2s[sub], eps_sb)
                    x2 = opool.tile([128, D], F32, tag="x2")
                    if apply_affine:
                        nc.vector.tensor_scalar(
                            x2[:], y2s[sub][:], mv[:, 0:1], rstd[:, 0:1],
                            ALU.subtract, ALU.mult)
                        nc.vector.scalar_tensor_tensor(
                            x2[:], x2[:], 1.0, g2_sb[:], ALU.mult, ALU.mult)
                        nc.vector.tensor_add(x2[:], x2[:], be2_sb[:])
                    else:
                        nc.scalar.activation(x2[:], y2s[sub][:],
                                             AF.Identity,
                                             bias=nmr[:, 0:1],
                                             scale=rstd[:, 0:1])
                    nc.sync.dma_start(
                        out_q[sub * 128:(sub + 1) * 128, :], x2[:])

    nc.compile()
    _BUILD_CACHE[apply_affine] = nc
    return nc


def _ln_mv(nc, pool, y, eps_sb):
    """bn_stats-based LN stats: returns (mv [mean|var], rstd) tiles."""
    st = pool.tile([128, 2, 6], F32, tag="ln_st")
    yv = y.rearrange("p (g f) -> p g f", g=2)
    nc.vector.bn_stats(st[:, 0, :], yv[:, 0, :])
    nc.vector.bn_stats(st[:, 1, :], yv[:, 1, :])
    mv = pool.tile([128, 2], F32, tag="ln_mv")
    nc.vector.bn_aggr(mv[:], st[:])
    std = pool.tile([128, 1], F32, tag="ln_std")
    nc.scalar.activation(std[:], mv[:, 1:2], AF.Sqrt,
                         bias=eps_sb[:, 0:1])
    rstd = pool.tile([128, 1], F32, tag="ln_rstd")
    nc.vector.reciprocal(rstd[:], std[:])
    nmr = pool.tile([128, 1], F32, tag="ln_nmr")
    nc.vector.tensor_scalar(nmr[:], mv[:, 0:1], rstd[:, 0:1], -1.0,
                            ALU.mult, ALU.mult)
    return mv, rstd, nmr, std


def kernel(x, Wq, bq, ln1_g, ln1_b, W1, b1, W2, b2, ln2_g, ln2_b):
    x = np.asarray(x, np.float32)
    f8 = dt.np(F8)
    trivial = (np.all(ln1_g == 1) and np.all(ln1_b == 0)
               and np.all(ln2_g == 1) and np.all(ln2_b == 0))
    nc = _build(apply_affine=not trivial)

    Wqf = np.asarray(Wq, np.float32).transpose(1, 0, 2).reshape(D, D)
    bf16 = dt.np(BF16)
    # fold the LN1 affine into W1 / the rank-1 correction rows
    W1g = np.asarray(W1, np.float32) * np.asarray(ln1_g, np.float32)[:, None]
    b1e = (np.asarray(b1, np.float32)
           + np.asarray(ln1_b, np.float32) @ np.asarray(W1, np.float32))
    c1 = np.stack([W1g.sum(0), WS * b1e])[None].astype(f8)
    base = {
        "wq_dr": np.ascontiguousarray(
            (WS * Wqf).reshape(4, 2, 128, D).transpose(0, 2, 1, 3)
        ).astype(f8),
        "bq_r": np.ascontiguousarray(
            np.asarray(bq, np.float32).reshape(8, 128).T),
        "w1_dr": np.ascontiguousarray(
            (WS * W1g)
            .reshape(4, 2, 128, HID).transpose(0, 2, 1, 3)).astype(f8),
        "c1_d": np.ascontiguousarray(c1),
        "w2_dr": np.ascontiguousarray(
            (WS * np.asarray(W2, np.float32))
            .reshape(16, 2, 128, D).transpose(0, 2, 1, 3)).astype(f8),
        "b2_bc": np.ascontiguousarray(
            np.broadcast_to(np.asarray(b2, np.float32), (128, D))),
    }
    if not trivial:
        for name, v in (("g1d", ln1_g), ("be1d", ln1_b),
                        ("g2d", ln2_g), ("be2d", ln2_b)):
            base[name] = np.ascontiguousarray(
                np.broadcast_to(np.asarray(v, np.float32), (128, D)))

    in_maps = []
    for c in range(NCORES):
        b, t = divmod(c, 4)
        xb = np.concatenate([x[b, t * SQ:], x[b, :t * SQ]], axis=0)
        in_maps.append({
            **base,
            "xT_dr": np.ascontiguousarray(
                xb.T.reshape(4, 2, 128, S).transpose(0, 2, 1, 3)
            ).astype(f8),
            "x_q": np.ascontiguousarray(xb[:SQ]),
        })

    import os
    trace = bool(int(os.environ.get("KERNEL_TRACE", "0")))
    kw = {}
    if trace:
        kw = dict(trace=True,
                  tmpdir=os.environ.get("KERNEL_TRACE_DIR") or None)
    res = run_bass_kernel_spmd(nc, in_maps, core_ids=list(range(NCORES)),
                               **kw)
    out = np.empty((B, S, D), np.float32)
    for c in range(NCORES):
        b, t = divmod(c, 4)
        out[b, t * SQ:(t + 1) * SQ] = res.results[c]["out_q"]
    return out


# revision 5
# speedup vs baseline: 1.0185x; 1.0072x over previous
"""Trainium2 Bass kernel for a dense transformer encoder layer (v2).

Reference semantics (B=2, S=2048, D=1024, H=16, DH=64, HID=4096):
    q = einsum('bsd,hde->bhse', x, Wq) + bq          (q == k == v, source bug)
    prob = softmax(q @ q^T / sqrt(DH))
    attn = concat_heads(prob @ q)
    x1 = LN(x + attn);  ff = relu(x1 @ W1 + b1) @ W2 + b2;  out = LN(x1 + ff)

Sharding: 8 cores, core c -> batch b=c//4, token quarter t=c%4.  Each core
computes q for the full (rotated) sequence of its batch, then attention +
FFN for its own 512 tokens.  Zero collectives; quarters reassembled on host.

Structure:
  - Host supplies x^T pre-packed in fp8 DoubleRow layout; q projection and
    both FFN matmuls run fp8e4 DoubleRow (weights pre-scaled x16, unscaled
    in the psum epilogues; h1 is kept at 16x so relu needs no extra op).
  - q-natural layout (wv moving operand) comes from on-chip PE transposes
    of qT; no DMA transposes or DRAM round trips anywhere.
  - wv: stationary = exp(scores) [keys x queries] chunk, moving = q-natural
    [keys x 64] plus a ones column for the softmax denominator, so head
    outputs land directly in [queries, dims] layout; epilogue is one
    reciprocal + scalar_tensor_tensor into y1 per (h, qc).
  - exp(scores/8 - 1) keeps E inside fp8e4 range; softmax cancels e^-1.
  - The ACT engine (exp) is the attention bottleneck; scores pace it via
    psum rotation while wv/qproj/transpose chunks fill PE between score
    matmuls.  DMAs are scheduled so xT/Wq land first and W1/W2 stream in
    mid-attention.  LayerNorm stats use bn_stats/bn_aggr on DVE.
"""

import numpy as np

import concourse.bacc as bacc
import concourse.mybir as mybir
from concourse import tile
from concourse.bass_utils import run_bass_kernel_spmd

dt = mybir.dt
AF = mybir.ActivationFunctionType
ALU = mybir.AluOpType
DR = mybir.MatmulPerfMode.DoubleRow

B, S, D = 2, 2048, 1024
H, DH, HID = 16, 64, 4096
SQ = S // 4
NCORES = 8
EPS = 1e-5
WS = 16.0          # fp8 weight pre-scale
F32, BF16, F8 = dt.float32, dt.bfloat16, dt.float8e4

_BUILD_CACHE = {}


def _build(apply_affine: bool):
    if apply_affine in _BUILD_CACHE:
        return _BUILD_CACHE[apply_affine]

    nc = bacc.Bacc("TRN2", target_bir_lowering=False, debug=False,
                   num_devices=NCORES)

    xT_dr = nc.dram_tensor("xT_dr", [128, 4, 2, S], F8,
                           kind="ExternalInput").ap()
    x_q = nc.dram_tensor("x_q", [SQ, D], F32, kind="ExternalInput").ap()
    wq_dr = nc.dram_tensor("wq_dr", [128, 4, 2, D], F8,
                           kind="ExternalInput").ap()
    bq_r = nc.dram_tensor("bq_r", [128, 8], F32, kind="ExternalInput").ap()
    w1_dr = nc.dram_tensor("w1_dr", [4, 128, 2, HID], F8,
                           kind="ExternalInput").ap()
    c1_d = nc.dram_tensor("c1_d", [1, 2, HID], F8,
                          kind="ExternalInput").ap()
    w2_dr = nc.dram_tensor("w2_dr", [16, 128, 2, D], F8,
                           kind="ExternalInput").ap()
    b2_bc = nc.dram_tensor("b2_bc", [128, D], F32, kind="ExternalInput").ap()
    if apply_affine:
        g1d = nc.dram_tensor("g1d", [128, D], F32, kind="ExternalInput").ap()
        be1d = nc.dram_tensor("be1d", [128, D], F32,
                              kind="ExternalInput").ap()
        g2d = nc.dram_tensor("g2d", [128, D], F32, kind="ExternalInput").ap()
        be2d = nc.dram_tensor("be2d", [128, D], F32,
                              kind="ExternalInput").ap()
    out_q = nc.dram_tensor("out_q", [SQ, D], F32, kind="ExternalOutput").ap()

    with tile.TileContext(nc) as tc:
        with (
            tc.tile_pool(name="const", bufs=1) as cpool,
            tc.tile_pool(name="wts", bufs=1) as wpool,
            tc.tile_pool(name="y1", bufs=4) as y1pool,
            tc.tile_pool(name="x1", bufs=1) as x1pool,
            tc.tile_pool(name="ln", bufs=2) as lnpool,
        ):
            # ---- small constants (cheap DMAs first, they are tiny) ----
            bq_sb = cpool.tile([128, 8], F32)
            nc.sync.dma_start(bq_sb[:], bq_r[:])
            c1_sb = cpool.tile([1, 2, HID], F8)
            nc.sync.dma_start(c1_sb[:], c1_d[:])

            eps_sb = cpool.tile([128, 1], F32)
            nc.vector.memset(eps_sb[:], EPS)
            neg1_sb = cpool.tile([128, 1], F32)
            nc.vector.memset(neg1_sb[:], -1.0)
            ones_f8 = cpool.tile([128, 1], F8)
            nc.vector.memset(ones_f8[:], 1.0)

            # fp8 identity for PE-mode transposes
            col_i = cpool.tile([128, 128], F32)
            nc.gpsimd.iota(col_i[:], [[1, 128]], channel_multiplier=0,
                           allow_small_or_imprecise_dtypes=True)
            row_i = cpool.tile([128, 1], F32)
            nc.gpsimd.iota(row_i[:], [[0, 1]], channel_multiplier=1,
                           allow_small_or_imprecise_dtypes=True)
            idn = cpool.tile([128, 128], BF16)
            nc.vector.tensor_scalar(idn[:], col_i[:], row_i[:, 0:1], None,
                                    ALU.is_equal)

            b2_sb = cpool.tile([128, D], F32)
            y1s = [y1pool.tile([128, D], F32, tag="y1", name=f"y1_{sub}")
                   for sub in range(4)]
            w1_sb = [wpool.tile([128, 2, HID], F8, tag=f"w1{k}",
                                name=f"w1_{k}") for k in range(4)]
            w2_sb = [wpool.tile([128, 2, D], F8, tag=f"w2{j}",
                                name=f"w2_{j}") for j in range(16)]
            if apply_affine:
                g1_sb = cpool.tile([128, D], F32)
                nc.sync.dma_start(g1_sb[:], g1d[:])
                be1_sb = cpool.tile([128, D], F32)
                nc.sync.dma_start(be1_sb[:], be1d[:])
                g2_sb = cpool.tile([128, D], F32)
                nc.sync.dma_start(g2_sb[:], g2d[:])
                be2_sb = cpool.tile([128, D], F32)
                nc.sync.dma_start(be2_sb[:], be2d[:])

            with (
                tc.tile_pool(name="qT", bufs=1) as qTpool,
                tc.tile_pool(name="qa", bufs=12) as qapool,
                tc.tile_pool(name="E", bufs=18) as Epool,
                tc.tile_pool(name="xw", bufs=1) as xwpool,
                tc.tile_pool(name="qps", bufs=1, space="PSUM") as qps,
                tc.tile_pool(name="scps", bufs=2, space="PSUM") as scps,
                tc.tile_pool(name="tps", bufs=1, space="PSUM") as tps,
                tc.tile_pool(name="uvps", bufs=2, space="PSUM") as uvps,
            ):
                # xT / Wq first: they gate the whole pipeline.
                # Packed [128, 4, 2, .] so each load is one big DMA.
                wq_sb = xwpool.tile([128, 4, 2, D], F8, tag="wq",
                                    name="wq_all")
                xT_sb = xwpool.tile([128, 4, 2, S], F8, tag="xT",
                                    name="xT_all")
                nc.sync.dma_start(wq_sb[:, :, :, 0:128],
                                  wq_dr[:, :, :, 0:128])
                nc.sync.dma_start(xT_sb[:, :, :, 0:512],
                                  xT_dr[:, :, :, 0:512])
                nc.sync.dma_start(xT_sb[:, :, :, 512:1024],
                                  xT_dr[:, :, :, 512:1024])
                nc.sync.dma_start(wq_sb[:, :, :, 128:D],
                                  wq_dr[:, :, :, 128:D])
                nc.sync.dma_start(xT_sb[:, :, :, 1024:S],
                                  xT_dr[:, :, :, 1024:S])
                qT = [qTpool.tile([128, S], BF16, tag=f"qT{p}",
                                  name=f"qT{p}") for p in range(8)]
                qa = [[None] * 4 for _ in range(8)]

                def emit_qproj(p):
                    """qT[p] = (x @ Wq[:, pair p])^T via DR matmuls + bias."""
                    for n in range(4):
                        ps = qps.tile([128, 512], F32, tag="qps",
                                      name=f"qps{p}_{n}")
                        for k in range(4):
                            nc.tensor.matmul(
                                ps[:],
                                wq_sb[:, k, :, p * 128:(p + 1) * 128],
                                xT_sb[:, k, :, n * 512:(n + 1) * 512],
                                start=(k == 0), stop=(k == 3), perf_mode=DR)
                        nc.vector.tensor_scalar(
                            qT[p][:, n * 512:(n + 1) * 512], ps[:],
                            1.0 / WS, bq_sb[:, p:p + 1], ALU.mult, ALU.add)

                def emit_qtrans(p):
                    """qa[p][g]: 4 chunks x [one|h0 d64|one|h1 d64] via PE
                    transpose.  The leading ones give the softmax
                    denominator as row 0 of the wv output."""
                    for g in range(4):
                        pt = tps.tile([128, 512], BF16, tag="tps",
                                      name=f"tps{p}_{g}")
                        for j in range(4):
                            c = 4 * g + j
                            nc.tensor.transpose(
                                pt[:, j * 128:(j + 1) * 128],
                                qT[p][:, c * 128:(c + 1) * 128], idn[:])
                        t = qapool.tile([128, 4, 2, 65], F8, tag="qa",
                                        name=f"qa{p}_{g}")
                        nc.gpsimd.memset(t[:, :, :, 0:1], 1.0)
                        nc.vector.tensor_copy(
                            t[:, :, :, 1:65],
                            pt[:].rearrange("p (c h d) -> p c h d",
                                            c=4, h=2))
                        qa[p][g] = t

                E = {}      # (h, kp) -> fp8 [128, 1024] tile

                def emit_scores_kp(h, kp):
                    p, half = h // 2, h % 2
                    rows = slice(half * 64, half * 64 + 64)
                    sc = scps.tile([128, 1024], F32, tag="sc",
                                   name=f"sc{h}_{kp}")
                    for kk in range(2):
                        c = 2 * kp + kk
                        nc.tensor.matmul(
                            sc[:, kk * 512:(kk + 1) * 512],
                            qT[p][rows, c * 128:(c + 1) * 128],
                            qT[p][rows, 0:512],
                            start=True, stop=True)
                    e = Epool.tile([128, 1024], F8, tag="E",
                                   name=f"E{h}_{kp}")
                    nc.scalar.activation(e[:], sc[:], AF.Exp, scale=0.125,
                                         bias=neg1_sb[:, 0:1])
                    E[(h, kp)] = e

                def emit_wv_qc(h, qc):
                    p, half = h // 2, h % 2
                    uv = uvps.tile([128, 65], F32, tag="uv",
                                   name=f"uv{h}_{qc}")
                    for kc in range(16):
                        est = E[(h, kc // 2)][
                            :, (kc % 2) * 512 + qc * 128:
                               (kc % 2) * 512 + (qc + 1) * 128]
                        qam = qa[p][kc // 4][:, kc % 4, half, :]
                        nc.tensor.matmul(uv[:], est, qam,
                                         start=(kc == 0), stop=(kc == 15))
                    rct = lnpool.tile([128, 1], F32, tag="rct", bufs=4,
                                      name=f"rct{h}_{qc}")
                    nc.vector.reciprocal(rct[:], uv[:, 0:1])
                    nc.vector.scalar_tensor_tensor(
                        y1s[qc][:, h * 64:(h + 1) * 64],
                        uv[:, 1:65], rct[:, 0:1],
                        y1s[qc][:, h * 64:(h + 1) * 64],
                        ALU.mult, ALU.add)

                # ---- attention pipeline, ACT(exp)-paced ----
                emit_qproj(0)
                emit_qtrans(0)
                # residual rows + FFN consts: after xT/wq on the DMA queue
                for sub in range(4):
                    nc.sync.dma_start(y1s[sub][:],
                                      x_q[sub * 128:(sub + 1) * 128, :])
                nc.sync.dma_start(b2_sb[:], b2_bc[:])

                for h in range(16):
                    nxt = h // 2 + 1
                    for kp in range(8):
                        emit_scores_kp(h, kp)
                        if h >= 1 and 1 <= kp <= 4:
                            emit_wv_qc(h - 1, kp - 1)
                        if h % 2 == 0 and nxt < 8:
                            if kp == 5:
                                emit_qproj(nxt)
                            elif kp == 6:
                                emit_qtrans(nxt)
                    if h == 5:
                        for k in range(4):
                            nc.sync.dma_start(w1_sb[k][:], w1_dr[k])
                    if h == 9:
                        for j in range(16):
                            nc.sync.dma_start(w2_sb[j][:], w2_dr[j])
                for qc in range(4):
                    emit_wv_qc(15, qc)

            # ---- LN1 stats only (LN affine is folded into FFN1) ----
            x1pb = []
            mss = []
            rs256 = []
            for sub in range(4):
                mv, rstd, nmr, std = _ln_mv(nc, lnpool, y1s[sub], eps_sb)
                pb = x1pool.tile([128, D], F32, tag=f"x1pb_{sub}",
                                 name=f"x1pb_{sub}")
                # ms rows for the rank-1 LN correction: [-mu | std]
                ms = lnpool.tile([128, 2], BF16, tag="ln_ms", bufs=4,
                                 name=f"ms{sub}")
                nc.vector.tensor_scalar(ms[:, 0:1], mv[:, 0:1], -WS, None,
                                        ALU.mult)
                nc.vector.tensor_copy(ms[:, 1:2], std[:, 0:1])
                rs = lnpool.tile([128, 1], F32, tag="ln_rs", bufs=4,
                                 name=f"rs{sub}")
                nc.vector.tensor_scalar(rs[:], rstd[:], 1.0 / (WS * WS),
                                        None, ALU.mult)
                mss.append(ms)
                rs256.append(rs)
                if apply_affine:
                    t = lnpool.tile([128, D], F32, tag="x1t", bufs=2)
                    nc.vector.tensor_scalar(t[:], y1s[sub][:], mv[:, 0:1],
                                            rstd[:, 0:1], ALU.subtract,
                                            ALU.mult)
                    nc.vector.scalar_tensor_tensor(
                        pb[:], t[:], 1.0, g1_sb[:], ALU.mult, ALU.mult)
                    nc.vector.tensor_add(pb[:], pb[:], be1_sb[:])
                    nc.vector.tensor_add(pb[:], pb[:], b2_sb[:])
                else:
                    nc.gpsimd.tensor_scalar(pb[:], y1s[sub][:], mv[:, 0:1],
                                            rstd[:, 0:1], ALU.subtract,
                                            ALU.mult)
                    nc.gpsimd.tensor_add(pb[:], pb[:], b2_sb[:])
                x1pb.append(pb)

            # ---- x1^T in DR layout via PE transpose; FFN; LN2 ----
            with (
                tc.tile_pool(name="ffn", bufs=1) as ffpool,
                tc.tile_pool(name="hps", bufs=4, space="PSUM") as hps,
                tc.tile_pool(name="fps", bufs=4, space="PSUM") as fps,
                tc.tile_pool(name="out", bufs=2) as opool,
            ):
                x1dr = [ffpool.tile([128, 2, SQ], F8, tag=f"x1dr{kk}",
                                    name=f"x1dr{kk}") for kk in range(4)]
                idf = ffpool.tile([128, 128], F32, tag="idf")
                nc.vector.tensor_copy(idf[:], idn[:])
                for k in range(8):
                    ptf = hps.tile([128, 512], F32, tag="hps",
                                   name=f"xt{k}")
                    for sub in range(4):
                        nc.tensor.transpose(
                            ptf[:, sub * 128:(sub + 1) * 128],
                            y1s[sub][:, k * 128:(k + 1) * 128], idf[:])
                    if k % 2 == 0:
                        nc.vector.tensor_copy(x1dr[k // 2][:, 0, :],
                                              ptf[:])
                    else:
                        nc.scalar.copy(x1dr[k // 2][:, 1, :], ptf[:])
                # [-mu | std] rows transposed to [2, 512] for the rank-1 fix
                msT = ffpool.tile([1, 2, SQ], F8, tag="msT")
                ptf = hps.tile([128, 512], F32, tag="hps", name="mst")
                ptb = ptf[:].bitcast(BF16)
                for sub in range(4):
                    nc.tensor.transpose(
                        ptb[0:1, sub * 128:(sub + 1) * 128],
                        mss[sub][:, 0:1], idn[:])
                    nc.tensor.transpose(
                        ptb[0:1, 512 + sub * 128:512 + (sub + 1) * 128],
                        mss[sub][:, 1:2], idn[:])
                nc.vector.tensor_copy(
                    msT[0:1, :, :],
                    ptb[0:1, 0:1024].rearrange("p (a f) -> p a f", a=2))

                # FFN1: h1^T = relu(x1 @ W1*16 + b1*16), kept at 16x scale
                h1dr = [ffpool.tile([128, 2, SQ], F8, tag=f"h1dr{jj}",
                                    name=f"h1dr{jj}") for jj in range(16)]
                for j in range(32):
                    ps = hps.tile([128, 512], F32, tag="hps", name=f"h{j}")
                    for k in range(4):
                        nc.tensor.matmul(
                            ps[:], w1_sb[k][:, :, j * 128:(j + 1) * 128],
                            x1dr[k][:], start=(k == 0), stop=False,
                            perf_mode=DR)
                    nc.tensor.matmul(
                        ps[:], c1_sb[:, :, j * 128:(j + 1) * 128], msT[:],
                        start=False, stop=True, perf_mode=DR)
                    if j % 2 == 0:
                        nc.vector.tensor_scalar(
                            h1dr[j // 2][:, j % 2, :], ps[:],
                            0.0, None, ALU.max)
                    else:
                        nc.scalar.activation(
                            h1dr[j // 2][:, j % 2, :], ps[:], AF.Relu)

                # FFN2 + residual;  psum = 256*(h1 @ W2)
                y2s = []
                for tc_ in range(4):
                    y2 = y1pool.tile([128, D], F32, tag="y2",
                                     name=f"y2_{tc_}")
                    for dg in range(2):
                        ps = fps.tile([128, 512], F32, tag="fps",
                                      name=f"f{tc_}_{dg}")
                        for jj in range(16):
                            nc.tensor.matmul(
                                ps[:],
                                h1dr[jj][:, :, tc_ * 128:(tc_ + 1) * 128],
                                w2_sb[jj][:, :, dg * 512:(dg + 1) * 512],
                                start=(jj == 0), stop=(jj == 15),
                                perf_mode=DR)
                        nc.vector.scalar_tensor_tensor(
                            y2[:, dg * 512:(dg + 1) * 512], ps[:],
                            rs256[tc_][:, 0:1],
                            x1pb[tc_][:, dg * 512:(dg + 1) * 512],
                            ALU.mult, ALU.add)
                    y2s.append(y2)

                # LN2 -> out
                for sub in range(4):
                    mv, rstd, nmr, std = _ln_mv(nc, lnpool, y2s[sub], eps_sb)
                    x2 = opool.tile([128, D], F32, tag="x2")
                    if apply_affine:
                        nc.vector.tensor_scalar(
                            x2[:], y2s[sub][:], mv[:, 0:1], rstd[:, 0:1],
                            ALU.subtract, ALU.mult)
                        nc.vector.scalar_tensor_tensor(
                            x2[:], x2[:], 1.0, g2_sb[:], ALU.mult, ALU.mult)
                        nc.vector.tensor_add(x2[:], x2[:], be2_sb[:])
                    else:
                        nc.scalar.activation(x2[:], y2s[sub][:],
                                             AF.Identity,
                                             bias=nmr[:, 0:1],
                                             scale=rstd[:, 0:1])
                    nc.sync.dma_start(
                        out_q[sub * 128:(sub + 1) * 128, :], x2[:])

    nc.compile()
    _BUILD_CACHE[apply_affine] = nc
    return nc


def _ln_mv(nc, pool, y, eps_sb):
    """bn_stats-based LN stats: returns (mv [mean|var], rstd) tiles."""
    st = pool.tile([128, 2, 6], F32, tag="ln_st")
    yv = y.rearrange("p (g f) -> p g f", g=2)
    nc.vector.bn_stats(st[:, 0, :], yv[:, 0, :])
    nc.vector.bn_stats(st[:, 1, :], yv[:, 1, :])
    mv = pool.tile([128, 2], F32, tag="ln_mv")
    nc.vector.bn_aggr(mv[:], st[:])
    std = pool.tile([128, 1], F32, tag="ln_std")
    nc.scalar.activation(std[:], mv[:, 1:2], AF.Sqrt,
                         bias=eps_sb[:, 0:1])
    rstd = pool.tile([128, 1], F32, tag="ln_rstd")
    nc.vector.reciprocal(rstd[:], std[:])
    nmr = pool.tile([128, 1], F32, tag="ln_nmr")
    nc.vector.tensor_scalar(nmr[:], mv[:, 0:1], rstd[:, 0:1], -1.0,
                            ALU.mult, ALU.mult)
    return mv, rstd, nmr, std


def kernel(x, Wq, bq, ln1_g, ln1_b, W1, b1, W2, b2, ln2_g, ln2_b):
    x = np.asarray(x, np.float32)
    f8 = dt.np(F8)
    trivial = (np.all(ln1_g == 1) and np.all(ln1_b == 0)
               and np.all(ln2_g == 1) and np.all(ln2_b == 0))
    nc = _build(apply_affine=not trivial)

    Wqf = np.asarray(Wq, np.float32).transpose(1, 0, 2).reshape(D, D)
    bf16 = dt.np(BF16)
    # fold the LN1 affine into W1 / the rank-1 correction rows
    W1g = np.asarray(W1, np.float32) * np.asarray(ln1_g, np.float32)[:, None]
    b1e = (np.asarray(b1, np.float32)
           + np.asarray(ln1_b, np.float32) @ np.asarray(W1, np.float32))
    c1 = np.stack([W1g.sum(0), WS * b1e])[None].astype(f8)
    base = {
        "wq_dr": np.ascontiguousarray(
            (WS * Wqf).reshape(4, 2, 128, D).transpose(2, 0, 1, 3)
        ).astype(f8),
        "bq_r": np.ascontiguousarray(
            np.asarray(bq, np.float32).reshape(8, 128).T),
        "w1_dr": np.ascontiguousarray(
            (WS * W1g)
            .reshape(4, 2, 128, HID).transpose(0, 2, 1, 3)).astype(f8),
        "c1_d": np.ascontiguousarray(c1),
        "w2_dr": np.ascontiguousarray(
            (WS * np.asarray(W2, np.float32))
            .reshape(16, 2, 128, D).transpose(0, 2, 1, 3)).astype(f8),
        "b2_bc": np.ascontiguousarray(
            np.broadcast_to(np.asarray(b2, np.float32), (128, D))),
    }
    if not trivial:
        for name, v in (("g1d", ln1_g), ("be1d", ln1_b),
                        ("g2d", ln2_g), ("be2d", ln2_b)):
            base[name] = np.ascontiguousarray(
                np.broadcast_to(np.asarray(v, np.float32), (128, D)))

    in_maps = []
    for c in range(NCORES):
        b, t = divmod(c, 4)
        xb = np.concatenate([x[b, t * SQ:], x[b, :t * SQ]], axis=0)
        in_maps.append({
            **base,
            "xT_dr": np.ascontiguousarray(
                xb.T.reshape(4, 2, 128, S).transpose(2, 0, 1, 3)
            ).astype(f8),
            "x_q": np.ascontiguousarray(xb[:SQ]),
        })

    import os
    trace = bool(int(os.environ.get("KERNEL_TRACE", "0")))
    kw = {}
    if trace:
        kw = dict(trace=True,
                  tmpdir=os.environ.get("KERNEL_TRACE_DIR") or None)
    res = run_bass_kernel_spmd(nc, in_maps, core_ids=list(range(NCORES)),
                               **kw)
    out = np.empty((B, S, D), np.float32)
    for c in range(NCORES):
        b, t = divmod(c, 4)
        out[b, t * SQ:(t + 1) * SQ] = res.results[c]["out_q"]
    return out


# revision 6
# speedup vs baseline: 1.0224x; 1.0038x over previous
"""Trainium2 Bass kernel for a dense transformer encoder layer (v2).

Reference semantics (B=2, S=2048, D=1024, H=16, DH=64, HID=4096):
    q = einsum('bsd,hde->bhse', x, Wq) + bq          (q == k == v, source bug)
    prob = softmax(q @ q^T / sqrt(DH))
    attn = concat_heads(prob @ q)
    x1 = LN(x + attn);  ff = relu(x1 @ W1 + b1) @ W2 + b2;  out = LN(x1 + ff)

Sharding: 8 cores, core c -> batch b=c//4, token quarter t=c%4.  Each core
computes q for the full (rotated) sequence of its batch, then attention +
FFN for its own 512 tokens.  Zero collectives; quarters reassembled on host.

Structure:
  - Host supplies x^T pre-packed in fp8 DoubleRow layout; q projection and
    both FFN matmuls run fp8e4 DoubleRow (weights pre-scaled x16, unscaled
    in the psum epilogues; h1 is kept at 16x so relu needs no extra op).
  - q-natural layout (wv moving operand) comes from on-chip PE transposes
    of qT; no DMA transposes or DRAM round trips anywhere.
  - wv: stationary = exp(scores) [keys x queries] chunk, moving = q-natural
    [keys x 64] plus a ones column for the softmax denominator, so head
    outputs land directly in [queries, dims] layout; epilogue is one
    reciprocal + scalar_tensor_tensor into y1 per (h, qc).
  - exp(scores/8 - 1) keeps E inside fp8e4 range; softmax cancels e^-1.
  - The ACT engine (exp) is the attention bottleneck; scores pace it via
    psum rotation while wv/qproj/transpose chunks fill PE between score
    matmuls.  DMAs are scheduled so xT/Wq land first and W1/W2 stream in
    mid-attention.  LayerNorm stats use bn_stats/bn_aggr on DVE.
"""

import numpy as np

import concourse.bacc as bacc
import concourse.mybir as mybir
from concourse import tile
from concourse.bass_utils import run_bass_kernel_spmd

dt = mybir.dt
AF = mybir.ActivationFunctionType
ALU = mybir.AluOpType
DR = mybir.MatmulPerfMode.DoubleRow

B, S, D = 2, 2048, 1024
H, DH, HID = 16, 64, 4096
SQ = S // 4
NCORES = 8
EPS = 1e-5
WS = 16.0          # fp8 weight pre-scale
F32, BF16, F8 = dt.float32, dt.bfloat16, dt.float8e4

_BUILD_CACHE = {}


def _build(apply_affine: bool):
    if apply_affine in _BUILD_CACHE:
        return _BUILD_CACHE[apply_affine]

    nc = bacc.Bacc("TRN2", target_bir_lowering=False, debug=False,
                   num_devices=NCORES)

    xT_dr = nc.dram_tensor("xT_dr", [128, 4, 2, S], F8,
                           kind="ExternalInput").ap()
    x_q = nc.dram_tensor("x_q", [SQ, D], F32, kind="ExternalInput").ap()
    wq_dr = nc.dram_tensor("wq_dr", [128, 4, 2, D], F8,
                           kind="ExternalInput").ap()
    bq_r = nc.dram_tensor("bq_r", [128, 8], F32, kind="ExternalInput").ap()
    w1_dr = nc.dram_tensor("w1_dr", [4, 128, 2, HID], F8,
                           kind="ExternalInput").ap()
    c1_d = nc.dram_tensor("c1_d", [1, 2, HID], F8,
                          kind="ExternalInput").ap()
    w2_dr = nc.dram_tensor("w2_dr", [16, 128, 2, D], F8,
                           kind="ExternalInput").ap()
    b2_bc = nc.dram_tensor("b2_bc", [128, D], F32, kind="ExternalInput").ap()
    if apply_affine:
        g1d = nc.dram_tensor("g1d", [128, D], F32, kind="ExternalInput").ap()
        be1d = nc.dram_tensor("be1d", [128, D], F32,
                              kind="ExternalInput").ap()
        g2d = nc.dram_tensor("g2d", [128, D], F32, kind="ExternalInput").ap()
        be2d = nc.dram_tensor("be2d", [128, D], F32,
                              kind="ExternalInput").ap()
    out_q = nc.dram_tensor("out_q", [SQ, D], F32, kind="ExternalOutput").ap()

    with tile.TileContext(nc) as tc:
        with (
            tc.tile_pool(name="const", bufs=1) as cpool,
            tc.tile_pool(name="wts", bufs=1) as wpool,
            tc.tile_pool(name="y1", bufs=4) as y1pool,
            tc.tile_pool(name="x1", bufs=1) as x1pool,
            tc.tile_pool(name="ln", bufs=2) as lnpool,
        ):
            # ---- small constants (cheap DMAs first, they are tiny) ----
            bq_sb = cpool.tile([128, 8], F32)
            nc.sync.dma_start(bq_sb[:], bq_r[:])
            c1_sb = cpool.tile([1, 2, HID], F8)
            nc.sync.dma_start(c1_sb[:], c1_d[:])

            eps_sb = cpool.tile([128, 1], F32)
            nc.vector.memset(eps_sb[:], EPS)
            neg1_sb = cpool.tile([128, 1], F32)
            nc.vector.memset(neg1_sb[:], -1.0)
            ones_f8 = cpool.tile([128, 1], F8)
            nc.vector.memset(ones_f8[:], 1.0)

            # fp8 identity for PE-mode transposes
            col_i = cpool.tile([128, 128], F32)
            nc.gpsimd.iota(col_i[:], [[1, 128]], channel_multiplier=0,
                           allow_small_or_imprecise_dtypes=True)
            row_i = cpool.tile([128, 1], F32)
            nc.gpsimd.iota(row_i[:], [[0, 1]], channel_multiplier=1,
                           allow_small_or_imprecise_dtypes=True)
            idn = cpool.tile([128, 128], BF16)
            nc.vector.tensor_scalar(idn[:], col_i[:], row_i[:, 0:1], None,
                                    ALU.is_equal)

            b2_sb = cpool.tile([128, D], F32)
            y1s = [y1pool.tile([128, D], F32, tag="y1", name=f"y1_{sub}")
                   for sub in range(4)]
            w1_sb = [wpool.tile([128, 2, HID], F8, tag=f"w1{k}",
                                name=f"w1_{k}") for k in range(4)]
            w2_sb = [wpool.tile([128, 2, D], F8, tag=f"w2{j}",
                                name=f"w2_{j}") for j in range(16)]
            if apply_affine:
                g1_sb = cpool.tile([128, D], F32)
                nc.sync.dma_start(g1_sb[:], g1d[:])
                be1_sb = cpool.tile([128, D], F32)
                nc.sync.dma_start(be1_sb[:], be1d[:])
                g2_sb = cpool.tile([128, D], F32)
                nc.sync.dma_start(g2_sb[:], g2d[:])
                be2_sb = cpool.tile([128, D], F32)
                nc.sync.dma_start(be2_sb[:], be2d[:])

            with (
                tc.tile_pool(name="qT", bufs=1) as qTpool,
                tc.tile_pool(name="qa", bufs=12) as qapool,
                tc.tile_pool(name="E", bufs=18) as Epool,
                tc.tile_pool(name="xw", bufs=1) as xwpool,
                tc.tile_pool(name="qps", bufs=1, space="PSUM") as qps,
                tc.tile_pool(name="scps", bufs=2, space="PSUM") as scps,
                tc.tile_pool(name="tps", bufs=1, space="PSUM") as tps,
                tc.tile_pool(name="uvps", bufs=2, space="PSUM") as uvps,
            ):
                # xT / Wq first: they gate the whole pipeline.
                # Packed [128, 4, 2, .] so each load is one big DMA.
                wq_sb = xwpool.tile([128, 4, 2, D], F8, tag="wq",
                                    name="wq_all")
                xT_sb = xwpool.tile([128, 4, 2, S], F8, tag="xT",
                                    name="xT_all")
                nc.sync.dma_start(wq_sb[:, :, :, 0:128],
                                  wq_dr[:, :, :, 0:128])
                nc.sync.dma_start(xT_sb[:, :, :, 0:512],
                                  xT_dr[:, :, :, 0:512])
                nc.sync.dma_start(xT_sb[:, :, :, 512:1024],
                                  xT_dr[:, :, :, 512:1024])
                nc.sync.dma_start(xT_sb[:, :, :, 1024:S],
                                  xT_dr[:, :, :, 1024:S])
                nc.sync.dma_start(wq_sb[:, :, :, 128:D],
                                  wq_dr[:, :, :, 128:D])
                qT = [qTpool.tile([128, S], BF16, tag=f"qT{p}",
                                  name=f"qT{p}") for p in range(8)]
                qa = [[None] * 4 for _ in range(8)]

                def emit_qproj(p):
                    """qT[p] = (x @ Wq[:, pair p])^T via DR matmuls + bias."""
                    for n in range(4):
                        ps = qps.tile([128, 512], F32, tag="qps",
                                      name=f"qps{p}_{n}")
                        for k in range(4):
                            nc.tensor.matmul(
                                ps[:],
                                wq_sb[:, k, :, p * 128:(p + 1) * 128],
                                xT_sb[:, k, :, n * 512:(n + 1) * 512],
                                start=(k == 0), stop=(k == 3), perf_mode=DR)
                        nc.vector.tensor_scalar(
                            qT[p][:, n * 512:(n + 1) * 512], ps[:],
                            1.0 / WS, bq_sb[:, p:p + 1], ALU.mult, ALU.add)

                def emit_qtrans(p):
                    """qa[p][g]: 4 chunks x [one|h0 d64|one|h1 d64] via PE
                    transpose.  The leading ones give the softmax
                    denominator as row 0 of the wv output."""
                    for g in range(4):
                        pt = tps.tile([128, 512], BF16, tag="tps",
                                      name=f"tps{p}_{g}")
                        for j in range(4):
                            c = 4 * g + j
                            nc.tensor.transpose(
                                pt[:, j * 128:(j + 1) * 128],
                                qT[p][:, c * 128:(c + 1) * 128], idn[:])
                        t = qapool.tile([128, 4, 2, 65], F8, tag="qa",
                                        name=f"qa{p}_{g}")
                        nc.gpsimd.memset(t[:, :, :, 0:1], 1.0)
                        nc.vector.tensor_copy(
                            t[:, :, :, 1:65],
                            pt[:].rearrange("p (c h d) -> p c h d",
                                            c=4, h=2))
                        qa[p][g] = t

                E = {}      # (h, kp) -> fp8 [128, 1024] tile

                def emit_scores_kp(h, kp):
                    p, half = h // 2, h % 2
                    rows = slice(half * 64, half * 64 + 64)
                    sc = scps.tile([128, 1024], F32, tag="sc",
                                   name=f"sc{h}_{kp}")
                    for kk in range(2):
                        c = 2 * kp + kk
                        nc.tensor.matmul(
                            sc[:, kk * 512:(kk + 1) * 512],
                            qT[p][rows, c * 128:(c + 1) * 128],
                            qT[p][rows, 0:512],
                            start=True, stop=True)
                    e = Epool.tile([128, 1024], F8, tag="E",
                                   name=f"E{h}_{kp}")
                    nc.scalar.activation(e[:], sc[:], AF.Exp, scale=0.125,
                                         bias=neg1_sb[:, 0:1])
                    E[(h, kp)] = e

                def emit_wv_qc(h, qc):
                    p, half = h // 2, h % 2
                    uv = uvps.tile([128, 65], F32, tag="uv",
                                   name=f"uv{h}_{qc}")
                    for kc in range(16):
                        est = E[(h, kc // 2)][
                            :, (kc % 2) * 512 + qc * 128:
                               (kc % 2) * 512 + (qc + 1) * 128]
                        qam = qa[p][kc // 4][:, kc % 4, half, :]
                        nc.tensor.matmul(uv[:], est, qam,
                                         start=(kc == 0), stop=(kc == 15))
                    rct = lnpool.tile([128, 1], F32, tag="rct", bufs=4,
                                      name=f"rct{h}_{qc}")
                    nc.vector.reciprocal(rct[:], uv[:, 0:1])
                    nc.vector.scalar_tensor_tensor(
                        y1s[qc][:, h * 64:(h + 1) * 64],
                        uv[:, 1:65], rct[:, 0:1],
                        y1s[qc][:, h * 64:(h + 1) * 64],
                        ALU.mult, ALU.add)

                # ---- attention pipeline, ACT(exp)-paced ----
                emit_qproj(0)
                # residual rows + FFN consts: after xT/wq on the DMA queue
                for sub in range(4):
                    nc.sync.dma_start(y1s[sub][:],
                                      x_q[sub * 128:(sub + 1) * 128, :])
                nc.sync.dma_start(b2_sb[:], b2_bc[:])

                for h in range(16):
                    nxt = h // 2 + 1
                    for kp in range(8):
                        emit_scores_kp(h, kp)
                        if h == 0 and kp == 2:
                            emit_qtrans(0)
                        if h >= 1 and 1 <= kp <= 4:
                            emit_wv_qc(h - 1, kp - 1)
                        if h % 2 == 0 and nxt < 8:
                            if kp == 5:
                                emit_qproj(nxt)
                            elif kp == 6:
                                emit_qtrans(nxt)
                    if h == 5:
                        for k in range(4):
                            nc.sync.dma_start(w1_sb[k][:], w1_dr[k])
                    if h == 9:
                        for j in range(16):
                            nc.sync.dma_start(w2_sb[j][:], w2_dr[j])
                for qc in range(4):
                    emit_wv_qc(15, qc)

            # ---- LN1 stats only (LN affine is folded into FFN1) ----
            x1pb = []
            mss = []
            rs256 = []
            for sub in range(4):
                mv, rstd, nmr, std = _ln_mv(nc, lnpool, y1s[sub], eps_sb)
                pb = x1pool.tile([128, D], F32, tag=f"x1pb_{sub}",
                                 name=f"x1pb_{sub}")
                # ms rows for the rank-1 LN correction: [-mu | std]
                ms = lnpool.tile([128, 2], BF16, tag="ln_ms", bufs=4,
                                 name=f"ms{sub}")
                nc.vector.tensor_scalar(ms[:, 0:1], mv[:, 0:1], -WS, None,
                                        ALU.mult)
                nc.vector.tensor_copy(ms[:, 1:2], std[:, 0:1])
                rs = lnpool.tile([128, 1], F32, tag="ln_rs", bufs=4,
                                 name=f"rs{sub}")
                nc.vector.tensor_scalar(rs[:], rstd[:], 1.0 / (WS * WS),
                                        None, ALU.mult)
                mss.append(ms)
                rs256.append(rs)
                if apply_affine:
                    t = lnpool.tile([128, D], F32, tag="x1t", bufs=2)
                    nc.vector.tensor_scalar(t[:], y1s[sub][:], mv[:, 0:1],
                                            rstd[:, 0:1], ALU.subtract,
                                            ALU.mult)
                    nc.vector.scalar_tensor_tensor(
                        pb[:], t[:], 1.0, g1_sb[:], ALU.mult, ALU.mult)
                    nc.vector.tensor_add(pb[:], pb[:], be1_sb[:])
                    nc.vector.tensor_add(pb[:], pb[:], b2_sb[:])
                else:
                    nc.gpsimd.tensor_scalar(pb[:], y1s[sub][:], mv[:, 0:1],
                                            rstd[:, 0:1], ALU.subtract,
                                            ALU.mult)
                    nc.gpsimd.tensor_add(pb[:], pb[:], b2_sb[:])
                x1pb.append(pb)

            # ---- x1^T in DR layout via PE transpose; FFN; LN2 ----
            with (
                tc.tile_pool(name="ffn", bufs=1) as ffpool,
                tc.tile_pool(name="hps", bufs=4, space="PSUM") as hps,
                tc.tile_pool(name="fps", bufs=4, space="PSUM") as fps,
                tc.tile_pool(name="out", bufs=2) as opool,
            ):
                x1dr = [ffpool.tile([128, 2, SQ], F8, tag=f"x1dr{kk}",
                                    name=f"x1dr{kk}") for kk in range(4)]
                idf = ffpool.tile([128, 128], F32, tag="idf")
                nc.vector.tensor_copy(idf[:], idn[:])
                for k in range(8):
                    ptf = hps.tile([128, 512], F32, tag="hps",
                                   name=f"xt{k}")
                    for sub in range(4):
                        nc.tensor.transpose(
                            ptf[:, sub * 128:(sub + 1) * 128],
                            y1s[sub][:, k * 128:(k + 1) * 128], idf[:])
                    if k % 2 == 0:
                        nc.vector.tensor_copy(x1dr[k // 2][:, 0, :],
                                              ptf[:])
                    else:
                        nc.scalar.copy(x1dr[k // 2][:, 1, :], ptf[:])
                # [-mu | std] rows transposed to [2, 512] for the rank-1 fix
                msT = ffpool.tile([1, 2, SQ], F8, tag="msT")
                ptf = hps.tile([128, 512], F32, tag="hps", name="mst")
                ptb = ptf[:].bitcast(BF16)
                for sub in range(4):
                    nc.tensor.transpose(
                        ptb[0:1, sub * 128:(sub + 1) * 128],
                        mss[sub][:, 0:1], idn[:])
                    nc.tensor.transpose(
                        ptb[0:1, 512 + sub * 128:512 + (sub + 1) * 128],
                        mss[sub][:, 1:2], idn[:])
                nc.vector.tensor_copy(
                    msT[0:1, :, :],
                    ptb[0:1, 0:1024].rearrange("p (a f) -> p a f", a=2))

                # FFN1: h1^T = relu(x1 @ W1*16 + b1*16), kept at 16x scale
                h1dr = [ffpool.tile([128, 2, SQ], F8, tag=f"h1dr{jj}",
                                    name=f"h1dr{jj}") for jj in range(16)]
                for j in range(32):
                    ps = hps.tile([128, 512], F32, tag="hps", name=f"h{j}")
                    for k in range(4):
                        nc.tensor.matmul(
                            ps[:], w1_sb[k][:, :, j * 128:(j + 1) * 128],
                            x1dr[k][:], start=(k == 0), stop=False,
                            perf_mode=DR)
                    nc.tensor.matmul(
                        ps[:], c1_sb[:, :, j * 128:(j + 1) * 128], msT[:],
                        start=False, stop=True, perf_mode=DR)
                    if j % 2 == 0:
                        nc.vector.tensor_scalar(
                            h1dr[j // 2][:, j % 2, :], ps[:],
                            0.0, None, ALU.max)
                    else:
                        nc.scalar.activation(
                            h1dr[j // 2][:, j % 2, :], ps[:], AF.Relu)

                # FFN2 + residual;  psum = 256*(h1 @ W2)
                y2s = []
                for tc_ in range(4):
                    y2 = y1pool.tile([128, D], F32, tag="y2",
                                     name=f"y2_{tc_}")
                    for dg in range(2):
                        ps = fps.tile([128, 512], F32, tag="fps",
                                      name=f"f{tc_}_{dg}")
                        for jj in range(16):
                            nc.tensor.matmul(
                                ps[:],
                                h1dr[jj][:, :, tc_ * 128:(tc_ + 1) * 128],
                                w2_sb[jj][:, :, dg * 512:(dg + 1) * 512],
                                start=(jj == 0), stop=(jj == 15),
                                perf_mode=DR)
                        nc.vector.scalar_tensor_tensor(
                            y2[:, dg * 512:(dg + 1) * 512], ps[:],
                            rs256[tc_][:, 0:1],
                            x1pb[tc_][:, dg * 512:(dg + 1) * 512],
                            ALU.mult, ALU.add)
                    y2s.append(y2)

                # LN2 -> out
                for sub in range(4):
                    mv, rstd, nmr, std = _ln_mv(nc, lnpool, y2s[sub], eps_sb)
                    x2 = opool.tile([128, D], F32, tag="x2")
                    if apply_affine:
                        nc.vector.tensor_scalar(
                            x2[:], y2s[sub][:], mv[:, 0:1], rstd[:, 0:1],
                            ALU.subtract, ALU.mult)
                        nc.vector.scalar_tensor_tensor(
                            x2[:], x2[:], 1.0, g2_sb[:], ALU.mult, ALU.mult)
                        nc.vector.tensor_add(x2[:], x2[:], be2_sb[:])
                    else:
                        nc.scalar.activation(x2[:], y2s[sub][:],
                                             AF.Identity,
                                             bias=nmr[:, 0:1],
                                             scale=rstd[:, 0:1])
                    nc.sync.dma_start(
                        out_q[sub * 128:(sub + 1) * 128, :], x2[:])

    nc.compile()
    _BUILD_CACHE[apply_affine] = nc
    return nc


def _ln_mv(nc, pool, y, eps_sb):
    """bn_stats-based LN stats: returns (mv [mean|var], rstd) tiles."""
    st = pool.tile([128, 2, 6], F32, tag="ln_st")
    yv = y.rearrange("p (g f) -> p g f", g=2)
    nc.vector.bn_stats(st[:, 0, :], yv[:, 0, :])
    nc.vector.bn_stats(st[:, 1, :], yv[:, 1, :])
    mv = pool.tile([128, 2], F32, tag="ln_mv")
    nc.vector.bn_aggr(mv[:], st[:])
    std = pool.tile([128, 1], F32, tag="ln_std")
    nc.scalar.activation(std[:], mv[:, 1:2], AF.Sqrt,
                         bias=eps_sb[:, 0:1])
    rstd = pool.tile([128, 1], F32, tag="ln_rstd")
    nc.vector.reciprocal(rstd[:], std[:])
    nmr = pool.tile([128, 1], F32, tag="ln_nmr")
    nc.vector.tensor_scalar(nmr[:], mv[:, 0:1], rstd[:, 0:1], -1.0,
                            ALU.mult, ALU.mult)
    return mv, rstd, nmr, std


def kernel(x, Wq, bq, ln1_g, ln1_b, W1, b1, W2, b2, ln2_g, ln2_b):
    x = np.asarray(x, np.float32)
    f8 = dt.np(F8)
    trivial = (np.all(ln1_g == 1) and np.all(ln1_b == 0)
               and np.all(ln2_g == 1) and np.all(ln2_b == 0))
    nc = _build(apply_affine=not trivial)

    Wqf = np.asarray(Wq, np.float32).transpose(1, 0, 2).reshape(D, D)
    bf16 = dt.np(BF16)
    # fold the LN1 affine into W1 / the rank-1 correction rows
    W1g = np.asarray(W1, np.float32) * np.asarray(ln1_g, np.float32)[:, None]
    b1e = (np.asarray(b1, np.float32)
           + np.asarray(ln1_b, np.float32) @ np.asarray(W1, np.float32))
    c1 = np.stack([W1g.sum(0), WS * b1e])[None].astype(f8)
    base = {
        "wq_dr": np.ascontiguousarray(
            (WS * Wqf).reshape(4, 2, 128, D).transpose(2, 0, 1, 3)
        ).astype(f8),
        "bq_r": np.ascontiguousarray(
            np.asarray(bq, np.float32).reshape(8, 128).T),
        "w1_dr": np.ascontiguousarray(
            (WS * W1g)
            .reshape(4, 2, 128, HID).transpose(0, 2, 1, 3)).astype(f8),
        "c1_d": np.ascontiguousarray(c1),
        "w2_dr": np.ascontiguousarray(
            (WS * np.asarray(W2, np.float32))
            .reshape(16, 2, 128, D).transpose(0, 2, 1, 3)).astype(f8),
        "b2_bc": np.ascontiguousarray(
            np.broadcast_to(np.asarray(b2, np.float32), (128, D))),
    }
    if not trivial:
        for name, v in (("g1d", ln1_g), ("be1d", ln1_b),
                        ("g2d", ln2_g), ("be2d", ln2_b)):
            base[name] = np.ascontiguousarray(
                np.broadcast_to(np.asarray(v, np.float32), (128, D)))

    in_maps = []
    for c in range(NCORES):
        b, t = divmod(c, 4)
        xb = np.concatenate([x[b, t * SQ:], x[b, :t * SQ]], axis=0)
        in_maps.append({
            **base,
            "xT_dr": np.ascontiguousarray(
                xb.T.reshape(4, 2, 128, S).transpose(2, 0, 1, 3)
            ).astype(f8),
            "x_q": np.ascontiguousarray(xb[:SQ]),
        })

    import os
    trace = bool(int(os.environ.get("KERNEL_TRACE", "0")))
    kw = {}
    if trace:
        kw = dict(trace=True,
                  tmpdir=os.environ.get("KERNEL_TRACE_DIR") or None)
    res = run_bass_kernel_spmd(nc, in_maps, core_ids=list(range(NCORES)),
                               **kw)
    out = np.empty((B, S, D), np.float32)
    for c in range(NCORES):
        b, t = divmod(c, 4)
        out[b, t * SQ:(t + 1) * SQ] = res.results[c]["out_q"]
    return out
